# revision 1
# baseline (speedup 1.0000x reference)
"""Trainium2 Bass kernel for the correlation-map embedding module.

Math (per (b, nf) pair):
  f1d = bilinear_down28(feature_i[b, nf])                  # [C, 28, 28]
  f2sel[c, k] = bilinear sample of feature_j[b, nf] at the K knn grid points
  corr[k, :, :] = relu(sum_c f2sel[c, k] * f1d[c, :, :])   # [K, 28, 28]
  out[k] = corr[k] / sum_hw(exp(corr[k])) * 10

Key restructurings vs the reference:
  - only the K=128 selected query positions of f2 are ever computed (4-tap
    weighted gather: ap_gather on GPSIMD + weighting on DVE + the tap
    reduction folded into 4 accumulating matmuls), not the full 784 grid;
  - bilinear taps are exactly (2k, 2k+1) per output index, so the f1
    downsample is a single fused 4-tap weighted sum at 28x28 resolution:
    7 DVE ops on 784-elem tiles with precomputed product-weight planes;
  - the channel contraction runs on the tensor engine in float32r
    (full PE rate, ~1e-3 relative accuracy wrt fp32);
  - epilogue scaling rides the ScalarE activations: relu(corr)*10 via
    activation scale, exp(relu(corr)) via Exp with scale=0.1, final
    normalize via Copy with per-partition scale = 1/denom.

Sharding: pure data parallel — batch dim (16) split across 8 cores, 2 each.
"""

import numpy as np

# hardcoded problem shapes (grading calls kernel(**inputs) standalone)
B, NF, C, H, W = 16, 3, 128, 56, 56
G = 28
K = 128
NCORES = 8
BPC = B // NCORES  # 2
P = 128

_CACHE = {}


def _axis_coords(n_in):
    # float32 arithmetic to match the jax reference bit-for-bit
    src = np.arange(G, dtype=np.float32) * np.float32((n_in - 1) / (G - 1))
    i0 = np.clip(np.floor(src).astype(np.int32), 0, n_in - 2)
    w = (src - i0.astype(np.float32)).astype(np.float32)
    return i0, w


def _host_consts(knn_inds):
    i0h, wh = _axis_coords(H)
    i0w, ww = _axis_coords(W)
    # the even/odd strided-AP downsample assumes taps are (2k, 2k+1)
    assert np.array_equal(i0h, 2 * np.arange(G)) and np.array_equal(i0w, 2 * np.arange(G))

    # fused 4-tap downsample product-weight planes, each [28*28]
    # tap order (u, t): u = H-axis tap, t = W-axis tap
    ah, bh = (1.0 - wh), wh
    aw, bw = (1.0 - ww), ww
    w4 = np.stack(
        [
            np.outer(ah, aw).reshape(-1),
            np.outer(ah, bw).reshape(-1),
            np.outer(bh, aw).reshape(-1),
            np.outer(bh, bw).reshape(-1),
        ]
    ).astype(np.float32)  # [4, 784]

    # gather indices/weights for the 4 bilinear taps of each knn point
    knn = np.asarray(knn_inds).astype(np.int64)  # [NF, K, 2]
    gidx = np.zeros((NF, P, 16), dtype=np.int16)
    gidx2 = np.zeros((NF, P, 32), dtype=np.int16)
    gwts = np.zeros((NF, 4 * K), dtype=np.float32)
    for nf in range(NF):
        h2 = knn[nf, :, 1]
        w2 = knn[nf, :, 0]
        r0 = i0h[h2]
        c0 = i0w[w2]
        # d=2 gather: each index fetches the horizontally-contiguous tap pair
        # (r_u*W + c0, +1); index is in units of 2 elements (c0 even).
        # j = k*2 + u ordering: gathered tile is [P, K, 2, 2] = [P, K, 4]
        pos = np.stack(
            [(r0 * W + c0) // 2, ((r0 + 1) * W + c0) // 2], axis=1
        ).reshape(-1)  # [256]
        wt = np.stack(
            [ah[h2] * aw[w2], ah[h2] * bw[w2], bh[h2] * aw[w2], bh[h2] * bw[w2]],
            axis=1,
        ).reshape(-1)
        gwts[nf] = wt.astype(np.float32)
        # ap_gather index layout: gathered index j comes from partition j%16,
        # slot j//16 of its 16-partition group; replicate across the 8 groups
        wrapped = pos.reshape(16, 16).T.astype(np.int16)  # [16, 16]
        gidx[nf] = np.tile(wrapped, (8, 1))
        # merged variant: one gather per nf covering both batches stacked in
        # one [P, 2*H*W] tile; j = b*256 + k*2 + u, b offset in d=2 units
        pos2 = np.concatenate([pos, pos + H * W // 2])  # [512]
        wrapped2 = pos2.reshape(32, 16).T.astype(np.int16)  # [16, 32]
        gidx2[nf] = np.tile(wrapped2, (8, 1))
    return w4, gidx, gidx2, gwts


def _build_bass(repeat=1, mode="full"):
    """mode: "full" = real kernel; "dma" = only the DMA traffic (roofline probe).
    repeat: clone the whole per-pair pipeline R times (idempotent) so HW time
    can be measured by differencing two R values."""
    import concourse.bacc as bacc
    import concourse.tile as tile
    from concourse import mybir

    f32 = mybir.dt.float32
    f32r = mybir.dt.float32r
    i16 = mybir.dt.int16
    AF = mybir.ActivationFunctionType
    ALU = mybir.AluOpType

    nc = bacc.Bacc()
    fi = nc.dram_tensor("fi", [BPC, NF, C, H, W], f32, kind="ExternalInput")
    fj = nc.dram_tensor("fj", [BPC, NF, C, H, W], f32, kind="ExternalInput")
    w4_d = nc.dram_tensor("w4", [1, 4 * G * G + P], f32r, kind="ExternalInput")
    gidx_d = nc.dram_tensor("gidx", [NF, P, 16], i16, kind="ExternalInput")
    gidx2_d = nc.dram_tensor("gidx2", [NF, P, 32], i16, kind="ExternalInput")
    gw_d = nc.dram_tensor("gw", [1, NF * 4 * K], f32r, kind="ExternalInput")
    out_d = nc.dram_tensor("out", [BPC, NF, K, G, G], f32, kind="ExternalOutput")

    with tile.TileContext(nc) as tc:
        with (
            tc.tile_pool(name="consts", bufs=1) as consts,
            tc.tile_pool(name="feat2x", bufs=2) as feat2x,
            tc.tile_pool(name="feat1", bufs=2) as feat1,
            tc.tile_pool(name="work", bufs=2) as work,
            tc.tile_pool(name="psum", bufs=3, space="PSUM") as pspool,
            tc.tile_pool(name="bcpsum", bufs=2, space="PSUM") as bcpool,
            tc.tile_pool(name="outp", bufs=3) as outp,
        ):
            # constants: load single-partition rows from HBM (tiny), then
            # replicate across partitions with ones-vector matmuls on the idle
            # PE — avoids 2.3MB of broadcast DMA traffic on the memory-bound
            # critical path. float32r rounding of the weights (~1e-3) is in
            # the same class as the correlation matmul's own rounding.
            w4row = consts.tile([1, 4 * G * G + P], f32r, tag="w4row")
            nc.scalar.dma_start(out=w4row, in_=w4_d[:, :])
            gwrow = consts.tile([1, NF * 4 * K], f32r, tag="gwrow")
            nc.scalar.dma_start(out=gwrow, in_=gw_d[:, :])
            # trailing P entries of the w4 input are 1.0: the ones row for
            # the PE partition-broadcast matmuls
            ones = w4row[:, 4 * G * G : 4 * G * G + P]

            def pe_broadcast(row_ap, n):
                """[1, n] -> [P, n] via PE: out = ones.T @ row."""
                dst = consts.tile([P, n], f32, tag=f"bc{len(bc_tiles)}")
                done = 0
                while done < n:
                    chunk = min(512, n - done)
                    bps = bcpool.tile([P, 512], f32, tag="bps")
                    nc.tensor.matmul(
                        bps[:, :chunk],
                        lhsT=ones,
                        rhs=row_ap[:, done : done + chunk],
                        start=True,
                        stop=True,
                    )
                    nc.scalar.copy(dst[:, done : done + chunk], bps[:, :chunk])
                    done += chunk
                bc_tiles.append(dst)
                return dst

            bc_tiles = []
            w4_t = [
                pe_broadcast(w4row[:, u * G * G : (u + 1) * G * G], G * G)
                for u in range(4)
            ]
            gw_t = [
                pe_broadcast(gwrow[:, nf * 4 * K : (nf + 1) * 4 * K], 4 * K)
                for nf in range(NF)
            ]
            gidx_t = []
            gidx2_t = []
            for nf in range(NF):
                it = consts.tile([P, 16], i16, tag=f"gidx{nf}")
                nc.scalar.dma_start(out=it, in_=gidx_d[nf])
                gidx_t.append(it)
                it2 = consts.tile([P, 32], i16, tag=f"gidx2_{nf}")
                nc.scalar.dma_start(out=it2, in_=gidx2_d[nf])
                gidx2_t.append(it2)

            import contextlib

            loop_ctx = (
                tc.For_i(0, repeat, 1) if repeat > 1 else contextlib.nullcontext()
            )
            HH = H // 2  # 28 rows per half
            GH = G // 2  # 14 output rows per half
            merged = mode in ("full", "dma")
            with loop_ctx:
              for nf in range(NF):
                # f1 halves for both batches FIRST: the downsample (the bulk
                # of DVE work) streams while f2 is still loading, so no DVE
                # backlog trails the last DMA
                f1hs = {}
                for b in range(BPC):
                    f1hs[b] = []
                    for h in range(2):
                        t = feat1.tile([P, HH, W], f32, tag=f"f1h{b}_{h}")
                        nc.sync.dma_start(
                            out=t, in_=fi[b, nf, :, h * HH : (h + 1) * HH, :]
                        )
                        f1hs[b].append(t)
                if merged and mode != "dma":
                    # both batches' f2 stacked in one tile; single gather per
                    # nf amortizes the ap_gather fixed cost
                    f2x = feat2x.tile([P, BPC, H * W], f32, tag="f2x")
                    for b in range(BPC):
                        nc.sync.dma_start(
                            out=f2x[:, b, :],
                            in_=fj[b, nf].rearrange("p h w -> p (h w)"),
                        )
                    g2 = work.tile([P, BPC, K, 4], f32, tag="g2")
                    nc.gpsimd.ap_gather(
                        g2.rearrange("p b k t -> p (b k t)"),
                        f2x.rearrange("p b q -> p (b q)"),
                        gidx2_t[nf],
                        channels=P,
                        num_elems=BPC * H * W // 2,
                        d=2,
                        num_idxs=BPC * 2 * K,
                    )
                for b in range(BPC):
                    f1h = f1hs[b]
                    if not merged:
                        f2 = feat2x.tile([P, H, W], f32, tag="f2")
                        nc.sync.dma_start(out=f2, in_=fj[b, nf])

                    if mode == "dma":
                        # inputs: fj loaded once per (b, nf) like full
                        f2 = feat2x.tile([P, H, W], f32, tag="f2")
                        nc.sync.dma_start(out=f2, in_=fj[b, nf])
                        o = outp.tile([P, G * G], f32, tag="o")
                        nc.vector.memset(o, 0.0)
                        nc.scalar.dma_start(
                            out=out_d[b, nf].rearrange("k g1 g2 -> k (g1 g2)"), in_=o
                        )
                        continue

                    # f2 at the K selected grid points: gather the 4 bilinear
                    # taps (GPSIMD), apply tap weights (DVE)
                    if merged:
                        g = g2[:, b]
                    else:
                        g = work.tile([P, K, 4], f32, tag="g")
                        if mode == "nogather":
                            nc.vector.tensor_copy(
                                g.rearrange("p k t -> p (k t)"),
                                f2.rearrange("p h w -> p (h w)")[:, : 4 * K],
                            )
                        else:
                            nc.gpsimd.ap_gather(
                                g.rearrange("p k t -> p (k t)"),
                                f2.rearrange("p h w -> p (h w)"),
                                gidx_t[nf],
                                channels=P,
                                num_elems=H * W // 2,
                                d=2,
                                num_idxs=2 * K,
                            )
                    gg = work.tile([P, K, 4], f32r, tag="gg")
                    nc.vector.tensor_mul(
                        gg.rearrange("p k t -> p (k t)"),
                        g.rearrange("p k t -> p (k t)"),
                        gw_t[nf],
                    )

                    # per half: fused 4-tap downsample (DVE) + 4 accumulating
                    # matmuls; corr[k, q] = sum_c sum_t gg[c,k,t] * f1d[c,q]
                    ps = pspool.tile([P, 2, 512], f32, tag="ps")
                    for h in range(2):
                        f1v = f1h[h].rearrange(
                            "p (h uu) (w tt) -> p h uu w tt", uu=2, tt=2
                        )
                        m = []
                        for u in range(2):
                            for t in range(2):
                                mt = work.tile([P, GH, G], f32, tag=f"m{u}{t}")
                                nc.vector.tensor_mul(
                                    mt,
                                    f1v[:, :, u, :, t],
                                    w4_t[2 * u + t].rearrange(
                                        "p (h g) -> p h g", g=G
                                    )[:, h * GH : (h + 1) * GH, :],
                                )
                                m.append(mt)
                        a0 = work.tile([P, GH, G], f32, tag="a0")
                        nc.vector.tensor_add(a0, m[0], m[1])
                        a1 = work.tile([P, GH, G], f32, tag="a1")
                        nc.vector.tensor_add(a1, m[2], m[3])
                        f1d = work.tile([P, GH, G], f32r, tag="f1d")
                        nc.vector.tensor_add(f1d, a0, a1)

                        rhs = f1d.rearrange("p h g -> p (h g)")  # [P, 392]
                        for t in range(4):
                            nc.tensor.matmul(
                                ps[:, h, : GH * G],
                                lhsT=gg[:, :, t],
                                rhs=rhs,
                                start=(t == 0),
                                stop=(t == 3),
                            )

                    # epilogue on ScalarE: r = 10*relu(corr); s = sum(exp(r/10));
                    # out = r * (1/s)
                    r = outp.tile([P, 2, GH * G], f32, tag="r")
                    nc.scalar.activation(r, ps[:, :, : GH * G], AF.Relu, scale=10.0)
                    rf = r.rearrange("p h q -> p (h q)")  # [P, 784] contiguous
                    e = work.tile([P, G * G], f32, tag="e")
                    s = work.tile([P, 1], f32, tag="s")
                    nc.scalar.activation(e, rf, AF.Exp, scale=0.1, accum_out=s)
                    rec = work.tile([P, 1], f32, tag="rec")
                    nc.vector.reciprocal(rec, s)
                    o = outp.tile([P, G * G], f32, tag="o")
                    # final normalize on DVE (tensor_scalar runs in 2x mode)
                    nc.vector.tensor_scalar(
                        o, rf, rec, None, op0=ALU.mult
                    )
                    # issue the store from ScalarE (mostly idle): keeps the
                    # SP/sync stream free to prefetch later pairs instead of
                    # stalling on this pair's compute chain
                    nc.scalar.dma_start(
                        out=out_d[b, nf].rearrange("k g1 g2 -> k (g1 g2)"), in_=o
                    )
    return nc


def _get_bass():
    if "nc" not in _CACHE:
        nc = _build_bass()
        # run the Bacc passes (reg alloc, library-load insertion) before the
        # PJRT path serializes the module
        if not nc.is_finalized():
            nc.finalize()
        _CACHE["nc"] = nc
    return _CACHE["nc"]


def kernel(feature_i, feature_j, mask, optical_flow, knn_inds):
    from concourse import bass_utils

    nc = _get_bass()
    w4, gidx, gidx2, gwts = _host_consts(knn_inds)

    fi = np.ascontiguousarray(np.asarray(feature_i, dtype=np.float32))
    fj = np.ascontiguousarray(np.asarray(feature_j, dtype=np.float32))

    in_maps = []
    for core in range(NCORES):
        lo = core * BPC
        in_maps.append(
            {
                "fi": fi[lo : lo + BPC],
                "fj": fj[lo : lo + BPC],
                "w4": np.concatenate([w4.reshape(-1), np.ones(P, np.float32)])[None, :],
                "gidx": gidx,
                "gidx2": gidx2,
                "gw": gwts.reshape(1, -1),
            }
        )

    res = bass_utils.run_bass_kernel_spmd(nc, in_maps, core_ids=list(range(NCORES)))
    out = np.concatenate([res.results[c]["out"] for c in range(NCORES)], axis=0)
    return out.astype(np.float32)



# revision 2
# speedup vs baseline: 1.0819x; 1.0819x over previous
"""Trainium2 Bass kernel for the correlation-map embedding module.

Math (per (b, nf) pair):
  f1d = bilinear_down28(feature_i[b, nf])                  # [C, 28, 28]
  f2sel[c, k] = bilinear sample of feature_j[b, nf] at the K knn grid points
  corr[k, :, :] = relu(sum_c f2sel[c, k] * f1d[c, :, :])   # [K, 28, 28]
  out[k] = corr[k] / sum_hw(exp(corr[k])) * 10

Key restructurings vs the reference:
  - only the K=128 selected query positions of f2 are ever computed (4-tap
    gather on GPSIMD, tap weighting + tap reduction on DVE), so every matmul
    shares one stationary f2sel operand;
  - the f1 bilinear downsample never materializes: each input element of the
    56x56 plane contributes to exactly one 28x28 output cell with one product
    weight, so a single contiguous full-plane multiply (f1 * W4full, one DVE
    op per half) replaces the strided 4-tap mul/add tree, and the 2x2-block
    reduction folds into 4 accumulating matmuls whose rhs are the even/odd
    strided views of the weighted plane;
  - the channel contraction runs on the tensor engine in float32r;
  - epilogue on ScalarE: relu(corr)*10 via activation scale, exp via Exp with
    scale=0.1 + accum_out, normalize via DVE tensor_scalar;
  - per nf, f2 loads before f1 so the gather -> weight -> reduce chain runs
    while f1 is still streaming (the gather chain is the longest dependency).

Sharding: pure data parallel - batch dim (16) split across 8 cores, 2 each.
"""

import numpy as np

# hardcoded problem shapes (grading calls kernel(**inputs) standalone)
B, NF, C, H, W = 16, 3, 128, 56, 56
G = 28
K = 128
NCORES = 8
BPC = B // NCORES  # 2
P = 128
HH = H // 2  # 28 input rows per half
GH = G // 2  # 14 output rows per half

_CACHE = {}


def _axis_coords(n_in):
    # float32 arithmetic to match the jax reference bit-for-bit
    src = np.arange(G, dtype=np.float32) * np.float32((n_in - 1) / (G - 1))
    i0 = np.clip(np.floor(src).astype(np.int32), 0, n_in - 2)
    w = (src - i0.astype(np.float32)).astype(np.float32)
    return i0, w


def _host_consts(knn_inds):
    i0h, wh = _axis_coords(H)
    i0w, ww = _axis_coords(W)
    # the even/odd strided-AP downsample assumes taps are (2k, 2k+1)
    assert np.array_equal(i0h, 2 * np.arange(G)) and np.array_equal(i0w, 2 * np.arange(G))

    ah, bh = (1.0 - wh), wh
    aw, bw = (1.0 - ww), ww
    # full-plane product weights: input element (2h'+u, 2w'+t) belongs to
    # output cell (h', w') with weight wh_tap[u][h'] * ww_tap[t][w']
    whfull = np.empty(H, dtype=np.float32)
    whfull[0::2] = ah
    whfull[1::2] = bh
    wwfull = np.empty(W, dtype=np.float32)
    wwfull[0::2] = aw
    wwfull[1::2] = bw
    w4full = np.outer(whfull, wwfull).astype(np.float32).reshape(-1)  # [3136]

    # gather indices/weights for the 4 bilinear taps of each knn point
    knn = np.asarray(knn_inds).astype(np.int64)  # [NF, K, 2]
    gidx = np.zeros((NF, P, 16), dtype=np.int16)
    gwts = np.zeros((NF, 4 * K), dtype=np.float32)
    for nf in range(NF):
        h2 = knn[nf, :, 1]
        w2 = knn[nf, :, 0]
        r0 = i0h[h2]
        c0 = i0w[w2]
        # d=2 gather: each index fetches the horizontally-contiguous tap pair
        # (r_u*W + c0, +1); index is in units of 2 elements (c0 even).
        # j = k*2 + u ordering: gathered tile is [P, K, 2, 2] = [P, K, 4]
        pos = np.stack(
            [(r0 * W + c0) // 2, ((r0 + 1) * W + c0) // 2], axis=1
        ).reshape(-1)  # [256]
        wt = np.stack(
            [ah[h2] * aw[w2], ah[h2] * bw[w2], bh[h2] * aw[w2], bh[h2] * bw[w2]],
            axis=1,
        ).reshape(-1)
        gwts[nf] = wt.astype(np.float32)
        # ap_gather index layout: gathered index j comes from partition j%16,
        # slot j//16 of its 16-partition group; replicate across the 8 groups
        wrapped = pos.reshape(16, 16).T.astype(np.int16)  # [16, 16]
        gidx[nf] = np.tile(wrapped, (8, 1))
    return w4full, gidx, gwts


def _build_bass():
    import concourse.bacc as bacc
    import concourse.tile as tile
    from concourse import mybir

    f32 = mybir.dt.float32
    f32r = mybir.dt.float32r
    i16 = mybir.dt.int16
    AF = mybir.ActivationFunctionType
    ALU = mybir.AluOpType

    nc = bacc.Bacc()
    # fi declared f32r so the in-place weighted plane feeds the PE at full rate
    fi = nc.dram_tensor("fi", [BPC, NF, C, H, W], f32r, kind="ExternalInput")
    fj = nc.dram_tensor("fj", [BPC, NF, C, H, W], f32, kind="ExternalInput")
    w4_d = nc.dram_tensor("w4", [1, H * W + P], f32r, kind="ExternalInput")
    gidx_d = nc.dram_tensor("gidx", [NF, P, 16], i16, kind="ExternalInput")
    gw_d = nc.dram_tensor("gw", [1, NF * 4 * K], f32r, kind="ExternalInput")
    out_d = nc.dram_tensor("out", [BPC, NF, K, G, G], f32, kind="ExternalOutput")

    with tile.TileContext(nc) as tc:
        with (
            tc.tile_pool(name="consts", bufs=1) as consts,
            tc.tile_pool(name="feat2", bufs=2) as feat2,
            tc.tile_pool(name="feat1", bufs=2) as feat1,
            tc.tile_pool(name="work", bufs=2) as work,
            tc.tile_pool(name="psum", bufs=3, space="PSUM") as pspool,
            tc.tile_pool(name="bcpsum", bufs=2, space="PSUM") as bcpool,
            tc.tile_pool(name="outp", bufs=3) as outp,
        ):
            # constants: load single-partition rows from HBM (tiny), then
            # replicate across partitions with ones-vector matmuls on the idle
            # PE - avoids MBs of broadcast DMA traffic on the memory-bound
            # critical path. float32r rounding of the weights (~1e-3) is in
            # the same class as the correlation matmul's own rounding.
            w4row = consts.tile([1, H * W + P], f32r, tag="w4row")
            nc.scalar.dma_start(out=w4row, in_=w4_d[:, :])
            gwrow = consts.tile([1, NF * 4 * K], f32r, tag="gwrow")
            nc.scalar.dma_start(out=gwrow, in_=gw_d[:, :])
            # trailing P entries of the w4 input are 1.0: the ones row for
            # the PE partition-broadcast matmuls
            ones = w4row[:, H * W : H * W + P]

            bc_count = [0]

            def pe_broadcast(row_ap, n):
                """[1, n] -> [P, n] via PE: out = ones.T @ row."""
                dst = consts.tile([P, n], f32, tag=f"bc{bc_count[0]}")
                bc_count[0] += 1
                done = 0
                while done < n:
                    chunk = min(512, n - done)
                    bps = bcpool.tile([P, 512], f32, tag="bps")
                    nc.tensor.matmul(
                        bps[:, :chunk],
                        lhsT=ones,
                        rhs=row_ap[:, done : done + chunk],
                        start=True,
                        stop=True,
                    )
                    nc.scalar.copy(dst[:, done : done + chunk], bps[:, :chunk])
                    done += chunk
                return dst

            w4bc = pe_broadcast(w4row[:, : H * W], H * W)  # [P, 3136]
            gw_t = [
                pe_broadcast(gwrow[:, nf * 4 * K : (nf + 1) * 4 * K], 4 * K)
                for nf in range(NF)
            ]
            gidx_t = []
            for nf in range(NF):
                it = consts.tile([P, 16], i16, tag=f"gidx{nf}")
                nc.scalar.dma_start(out=it, in_=gidx_d[nf])
                gidx_t.append(it)

            for nf in range(NF):
                # f2 first: the gather -> weight -> reduce chain is the
                # longest dependency; it runs while f1 still streams
                f2t = {}
                for b in range(BPC):
                    t = feat2.tile([P, H * W], f32, tag=f"f2_{b}")
                    nc.sync.dma_start(
                        out=t, in_=fj[b, nf].rearrange("p h w -> p (h w)")
                    )
                    f2t[b] = t
                f1h = {}
                for b in range(BPC):
                    f1h[b] = []
                    for h in range(2):
                        t = feat1.tile([P, HH, W], f32r, tag=f"f1_{b}_{h}")
                        nc.sync.dma_start(
                            out=t, in_=fi[b, nf, :, h * HH : (h + 1) * HH, :]
                        )
                        f1h[b].append(t)

                # f2 at the K selected grid points: gather the 4 bilinear
                # taps (GPSIMD), tap weights (DVE), tap reduction (DVE) so
                # all matmuls share one stationary f2sel operand
                f2sel = {}
                for b in range(BPC):
                    g = work.tile([P, K, 4], f32, tag=f"g{b}")
                    nc.gpsimd.ap_gather(
                        g.rearrange("p k t -> p (k t)"),
                        f2t[b],
                        gidx_t[nf],
                        channels=P,
                        num_elems=H * W // 2,
                        d=2,
                        num_idxs=2 * K,
                    )
                    gg = work.tile([P, K, 4], f32r, tag=f"gg{b}")
                    nc.vector.tensor_mul(
                        gg.rearrange("p k t -> p (k t)"),
                        g.rearrange("p k t -> p (k t)"),
                        gw_t[nf],
                    )
                    fs = work.tile([P, K], f32r, tag=f"fs{b}")
                    with nc.allow_low_precision(reason="f32r is fp32-width"):
                        nc.vector.tensor_reduce(
                            fs, gg, axis=mybir.AxisListType.X, op=ALU.add
                        )
                    f2sel[b] = fs

                for b in range(BPC):
                    ps = pspool.tile([P, 2, 512], f32, tag="ps")
                    for h in range(2):
                        # fused tap weighting: one contiguous in-place mul
                        # replaces the 4-tap strided mul/add tree
                        fh = f1h[b][h].rearrange("p h w -> p (h w)")
                        nc.vector.tensor_mul(
                            fh, fh, w4bc[:, h * HH * W : (h + 1) * HH * W]
                        )
                        # 2x2-block reduction on the PE: 4 accumulating
                        # matmuls over the even/odd strided views
                        f1v = f1h[b][h].rearrange(
                            "p (h uu) (w tt) -> p h uu w tt", uu=2, tt=2
                        )
                        i = 0
                        for u in range(2):
                            for t in range(2):
                                nc.tensor.matmul(
                                    ps[:, h, : GH * G],
                                    lhsT=f2sel[b],
                                    rhs=f1v[:, :, u, :, t],
                                    start=(i == 0),
                                    stop=(i == 3),
                                )
                                i += 1

                    # epilogue on ScalarE: r = 10*relu(corr);
                    # s = sum(exp(r/10)); out = r * (1/s)
                    r = outp.tile([P, 2, GH * G], f32, tag="r")
                    nc.scalar.activation(r, ps[:, :, : GH * G], AF.Relu, scale=10.0)
                    rf = r.rearrange("p h q -> p (h q)")  # [P, 784] contiguous
                    e = outp.tile([P, G * G], f32, tag="e")
                    s = work.tile([P, 1], f32, tag="s")
                    nc.scalar.activation(e, rf, AF.Exp, scale=0.1, accum_out=s)
                    rec = work.tile([P, 1], f32, tag="rec")
                    nc.vector.reciprocal(rec, s)
                    o = outp.tile([P, G * G], f32, tag="o")
                    # final normalize on DVE (tensor_scalar runs in 2x mode)
                    nc.vector.tensor_scalar(o, rf, rec, None, op0=ALU.mult)
                    # issue the store from ScalarE (mostly idle): keeps the
                    # SP/sync stream free to prefetch later pairs
                    nc.scalar.dma_start(
                        out=out_d[b, nf].rearrange("k g1 g2 -> k (g1 g2)"), in_=o
                    )
    return nc


def _get_bass():
    if "nc" not in _CACHE:
        nc = _build_bass()
        # run the Bacc passes (reg alloc, library-load insertion) before the
        # PJRT path serializes the module
        if not nc.is_finalized():
            nc.finalize()
        _CACHE["nc"] = nc
    return _CACHE["nc"]


def kernel(feature_i, feature_j, mask, optical_flow, knn_inds):
    from concourse import bass_utils

    nc = _get_bass()
    w4full, gidx, gwts = _host_consts(knn_inds)

    fi = np.ascontiguousarray(np.asarray(feature_i, dtype=np.float32))
    fj = np.ascontiguousarray(np.asarray(feature_j, dtype=np.float32))
    w4in = np.concatenate([w4full, np.ones(P, np.float32)])[None, :]

    in_maps = []
    for core in range(NCORES):
        lo = core * BPC
        in_maps.append(
            {
                "fi": fi[lo : lo + BPC],
                "fj": fj[lo : lo + BPC],
                "w4": w4in,
                "gidx": gidx,
                "gw": gwts.reshape(1, -1),
            }
        )

    res = bass_utils.run_bass_kernel_spmd(nc, in_maps, core_ids=list(range(NCORES)))
    out = np.concatenate([res.results[c]["out"] for c in range(NCORES)], axis=0)
    return out.astype(np.float32)


# revision 5
# speedup vs baseline: 1.1059x; 1.0221x over previous
"""Trainium2 Bass kernel for the correlation-map embedding module.

Math (per (b, nf) pair):
  f1d = bilinear_down28(feature_i[b, nf])                  # [C, 28, 28]
  f2sel[c, k] = bilinear sample of feature_j[b, nf] at the K knn grid points
  corr[k, :, :] = relu(sum_c f2sel[c, k] * f1d[c, :, :])   # [K, 28, 28]
  out[k] = corr[k] / sum_hw(exp(corr[k])) * 10

Key restructurings vs the reference:
  - only the K=128 selected query positions of f2 are ever computed (4-tap
    gather on GPSIMD, tap weighting + tap reduction on DVE), so every matmul
    shares one stationary f2sel operand;
  - the f1 bilinear downsample never materializes: each input element of the
    56x56 plane contributes to exactly one 28x28 output cell with one product
    weight, so a single contiguous full-plane multiply (f1 * W4full, one DVE
    op per half) replaces the strided 4-tap mul/add tree, and the 2x2-block
    reduction folds into 4 accumulating matmuls whose rhs are the even/odd
    strided views of the weighted plane;
  - the channel contraction runs on the tensor engine in float32r;
  - epilogue on ScalarE: relu(corr)*10 via activation scale, exp via Exp with
    scale=0.1 + accum_out, normalize via DVE tensor_scalar;
  - per nf, f2 loads before f1 so the gather -> weight -> reduce chain runs
    while f1 is still streaming (the gather chain is the longest dependency).

Sharding: pure data parallel - batch dim (16) split across 8 cores, 2 each.
"""

import numpy as np

# hardcoded problem shapes (grading calls kernel(**inputs) standalone)
B, NF, C, H, W = 16, 3, 128, 56, 56
G = 28
K = 128
NCORES = 8
BPC = B // NCORES  # 2
P = 128
HH = H // 2  # 28 input rows per half
GH = G // 2  # 14 output rows per half

_CACHE = {}


def _axis_coords(n_in):
    # float32 arithmetic to match the jax reference bit-for-bit
    src = np.arange(G, dtype=np.float32) * np.float32((n_in - 1) / (G - 1))
    i0 = np.clip(np.floor(src).astype(np.int32), 0, n_in - 2)
    w = (src - i0.astype(np.float32)).astype(np.float32)
    return i0, w


def _host_consts(knn_inds):
    i0h, wh = _axis_coords(H)
    i0w, ww = _axis_coords(W)
    # the even/odd strided-AP downsample assumes taps are (2k, 2k+1)
    assert np.array_equal(i0h, 2 * np.arange(G)) and np.array_equal(i0w, 2 * np.arange(G))

    ah, bh = (1.0 - wh), wh
    aw, bw = (1.0 - ww), ww
    # full-plane product weights: input element (2h'+u, 2w'+t) belongs to
    # output cell (h', w') with weight wh_tap[u][h'] * ww_tap[t][w']
    whfull = np.empty(H, dtype=np.float32)
    whfull[0::2] = ah
    whfull[1::2] = bh
    wwfull = np.empty(W, dtype=np.float32)
    wwfull[0::2] = aw
    wwfull[1::2] = bw
    w4full = np.outer(whfull, wwfull).astype(np.float32).reshape(-1)  # [3136]

    # gather indices/weights for the 4 bilinear taps of each knn point
    knn = np.asarray(knn_inds).astype(np.int64)  # [NF, K, 2]
    gidx2 = np.zeros((NF, P, 32), dtype=np.int16)
    gwts = np.zeros((NF, 4 * K), dtype=np.float32)
    for nf in range(NF):
        h2 = knn[nf, :, 1]
        w2 = knn[nf, :, 0]
        r0 = i0h[h2]
        c0 = i0w[w2]
        # d=2 gather: each index fetches the horizontally-contiguous tap pair
        # (r_u*W + c0, +1); index is in units of 2 elements (c0 even).
        # j = k*2 + u ordering: gathered tile is [P, K, 2, 2] = [P, K, 4]
        pos = np.stack(
            [(r0 * W + c0) // 2, ((r0 + 1) * W + c0) // 2], axis=1
        ).reshape(-1)  # [256]
        wt = np.stack(
            [ah[h2] * aw[w2], ah[h2] * bw[w2], bh[h2] * aw[w2], bh[h2] * bw[w2]],
            axis=1,
        ).reshape(-1)
        gwts[nf] = wt.astype(np.float32)
        # ap_gather index layout: gathered index j comes from partition j%16,
        # slot j//16 of its 16-partition group; replicate across the 8 groups.
        # merged variant: one gather per nf covering both batches stacked in
        # one [P, 2*H*W] tile; j = b*256 + k*2 + u, b offset in d=2 units
        pos2 = np.concatenate([pos, pos + H * W // 2])  # [512]
        wrapped2 = pos2.reshape(32, 16).T.astype(np.int16)  # [16, 32]
        gidx2[nf] = np.tile(wrapped2, (8, 1))
    return w4full, gidx2, gwts


def _build_bass():
    import concourse.bacc as bacc
    import concourse.tile as tile
    from concourse import mybir

    f32 = mybir.dt.float32
    f32r = mybir.dt.float32r
    i16 = mybir.dt.int16
    AF = mybir.ActivationFunctionType
    ALU = mybir.AluOpType

    nc = bacc.Bacc()
    # fi declared f32r so the in-place weighted plane feeds the PE at full rate
    fi = nc.dram_tensor("fi", [BPC, NF, C, H, W], f32r, kind="ExternalInput")
    fj = nc.dram_tensor("fj", [BPC, NF, C, H, W], f32, kind="ExternalInput")
    w4_d = nc.dram_tensor("w4", [1, H * W + P], f32r, kind="ExternalInput")
    gidx_d = nc.dram_tensor("gidx", [NF, P, 32], i16, kind="ExternalInput")
    gw_d = nc.dram_tensor("gw", [1, NF * 4 * K], f32r, kind="ExternalInput")
    out_d = nc.dram_tensor("out", [BPC, NF, K, G, G], f32, kind="ExternalOutput")

    with tile.TileContext(nc) as tc:
        with (
            tc.tile_pool(name="consts", bufs=1) as consts,
            tc.tile_pool(name="feat2", bufs=2) as feat2,
            tc.tile_pool(name="feat1", bufs=2) as feat1,
            tc.tile_pool(name="work", bufs=2) as work,
            tc.tile_pool(name="psum", bufs=3, space="PSUM") as pspool,
            tc.tile_pool(name="bcpsum", bufs=2, space="PSUM") as bcpool,
            tc.tile_pool(name="outp", bufs=3) as outp,
        ):
            # constants: load single-partition rows from HBM (tiny), then
            # replicate across partitions with ones-vector matmuls on the idle
            # PE - avoids MBs of broadcast DMA traffic on the memory-bound
            # critical path. float32r rounding of the weights (~1e-3) is in
            # the same class as the correlation matmul's own rounding.
            w4row = consts.tile([1, H * W + P], f32r, tag="w4row")
            nc.scalar.dma_start(out=w4row, in_=w4_d[:, :])
            gwrow = consts.tile([1, NF * 4 * K], f32r, tag="gwrow")
            nc.scalar.dma_start(out=gwrow, in_=gw_d[:, :])
            # trailing P entries of the w4 input are 1.0: the ones row for
            # the PE partition-broadcast matmuls
            ones = w4row[:, H * W : H * W + P]

            bc_count = [0]

            def pe_broadcast(row_ap, n):
                """[1, n] -> [P, n] via PE: out = ones.T @ row."""
                dst = consts.tile([P, n], f32, tag=f"bc{bc_count[0]}")
                bc_count[0] += 1
                done = 0
                while done < n:
                    chunk = min(512, n - done)
                    bps = bcpool.tile([P, 512], f32, tag="bps")
                    nc.tensor.matmul(
                        bps[:, :chunk],
                        lhsT=ones,
                        rhs=row_ap[:, done : done + chunk],
                        start=True,
                        stop=True,
                    )
                    nc.scalar.copy(dst[:, done : done + chunk], bps[:, :chunk])
                    done += chunk
                return dst

            w4bc = pe_broadcast(w4row[:, : H * W], H * W)  # [P, 3136]
            gw_t = [
                pe_broadcast(gwrow[:, nf * 4 * K : (nf + 1) * 4 * K], 4 * K)
                for nf in range(NF)
            ]
            gidx_t = []
            for nf in range(NF):
                it = consts.tile([P, 32], i16, tag=f"gidx{nf}")
                nc.scalar.dma_start(out=it, in_=gidx_d[nf])
                gidx_t.append(it)

            # dummy warmup gather: forces the GPSIMD ext-isa library load
            # (MODIFY_POOL_CONFIG + ~6us IRAM fetch) to happen during the
            # initial DMA fill instead of serializing the first real gather
            gdummy = consts.tile([P, 32], f32, tag="gdummy")
            nc.gpsimd.ap_gather(
                gdummy,
                w4bc[:, : H * W],
                gidx_t[0][:, :1],
                channels=P,
                num_elems=H * W // 2,
                d=2,
                num_idxs=16,
            )

            for nf in range(NF):
                # f2 first: the gather -> weight -> reduce chain is the
                # longest dependency; it runs while f1 still streams
                f2x = feat2.tile([P, BPC, H * W], f32, tag="f2x")
                for b in range(BPC):
                    nc.sync.dma_start(
                        out=f2x[:, b, :],
                        in_=fj[b, nf].rearrange("p h w -> p (h w)"),
                    )
                f1h = {}
                for b in range(BPC):
                    f1h[b] = []
                    for h in range(2):
                        t = feat1.tile([P, HH, W], f32r, tag=f"f1_{b}_{h}")
                        nc.sync.dma_start(
                            out=t, in_=fi[b, nf, :, h * HH : (h + 1) * HH, :]
                        )
                        f1h[b].append(t)

                # f2 at the K selected grid points: one merged gather per nf
                # (GPSIMD), then per-b tap weights + tap reduction (DVE) so
                # all matmuls share one stationary f2sel operand
                g2 = work.tile([P, BPC, K, 4], f32, tag="g2")
                nc.gpsimd.ap_gather(
                    g2.rearrange("p b k t -> p (b k t)"),
                    f2x.rearrange("p b q -> p (b q)"),
                    gidx_t[nf],
                    channels=P,
                    num_elems=BPC * H * W // 2,
                    d=2,
                    num_idxs=BPC * 2 * K,
                )

                # fused tap weighting first in DVE order: these only need f1
                # and run while the gather chain waits on f2
                for b in range(BPC):
                    for h in range(2):
                        fh = f1h[b][h].rearrange("p h w -> p (h w)")
                        nc.vector.tensor_mul(
                            fh, fh, w4bc[:, h * HH * W : (h + 1) * HH * W]
                        )

                f2sel = {}
                for b in range(BPC):
                    gg = work.tile([P, K, 4], f32r, tag=f"gg{b}")
                    nc.vector.tensor_mul(
                        gg.rearrange("p k t -> p (k t)"),
                        g2[:, b].rearrange("p k t -> p (k t)"),
                        gw_t[nf],
                    )
                    fs = work.tile([P, K], f32r, tag=f"fs{b}")
                    with nc.allow_low_precision(reason="f32r is fp32-width"):
                        nc.vector.tensor_reduce(
                            fs, gg, axis=mybir.AxisListType.X, op=ALU.add
                        )
                    f2sel[b] = fs

                for b in range(BPC):
                    ps = pspool.tile([P, 2, 512], f32, tag="ps")
                    for h in range(2):
                        # 2x2-block reduction on the PE: 4 accumulating
                        # matmuls over the even/odd strided views of the
                        # weighted plane
                        f1v = f1h[b][h].rearrange(
                            "p (h uu) (w tt) -> p h uu w tt", uu=2, tt=2
                        )
                        i = 0
                        for u in range(2):
                            for t in range(2):
                                nc.tensor.matmul(
                                    ps[:, h, : GH * G],
                                    lhsT=f2sel[b],
                                    rhs=f1v[:, :, u, :, t],
                                    start=(i == 0),
                                    stop=(i == 3),
                                )
                                i += 1

                    # epilogue on ScalarE: r = 10*relu(corr);
                    # s = sum(exp(r/10)); out = r * (1/s)
                    r = outp.tile([P, 2, GH * G], f32, tag="r")
                    nc.scalar.activation(r, ps[:, :, : GH * G], AF.Relu, scale=10.0)
                    rf = r.rearrange("p h q -> p (h q)")  # [P, 784] contiguous
                    e = outp.tile([P, G * G], f32, tag="e")
                    s = work.tile([P, 1], f32, tag="s")
                    nc.scalar.activation(e, rf, AF.Exp, scale=0.1, accum_out=s)
                    rec = work.tile([P, 1], f32, tag="rec")
                    nc.vector.reciprocal(rec, s)
                    o = outp.tile([P, G * G], f32, tag="o")
                    # final normalize on DVE (tensor_scalar runs in 2x mode)
                    nc.vector.tensor_scalar(o, rf, rec, None, op0=ALU.mult)
                    # issue the store from ScalarE (mostly idle): keeps the
                    # SP/sync stream free to prefetch later pairs
                    nc.scalar.dma_start(
                        out=out_d[b, nf].rearrange("k g1 g2 -> k (g1 g2)"), in_=o
                    )
    return nc


def _get_bass():
    if "nc" not in _CACHE:
        nc = _build_bass()
        # run the Bacc passes (reg alloc, library-load insertion) before the
        # PJRT path serializes the module
        if not nc.is_finalized():
            nc.finalize()
        _CACHE["nc"] = nc
    return _CACHE["nc"]


def kernel(feature_i, feature_j, mask, optical_flow, knn_inds):
    from concourse import bass_utils

    nc = _get_bass()
    w4full, gidx, gwts = _host_consts(knn_inds)

    fi = np.ascontiguousarray(np.asarray(feature_i, dtype=np.float32))
    fj = np.ascontiguousarray(np.asarray(feature_j, dtype=np.float32))
    w4in = np.concatenate([w4full, np.ones(P, np.float32)])[None, :]

    in_maps = []
    for core in range(NCORES):
        lo = core * BPC
        in_maps.append(
            {
                "fi": fi[lo : lo + BPC],
                "fj": fj[lo : lo + BPC],
                "w4": w4in,
                "gidx": gidx,
                "gw": gwts.reshape(1, -1),
            }
        )

    res = bass_utils.run_bass_kernel_spmd(nc, in_maps, core_ids=list(range(NCORES)))
    out = np.concatenate([res.results[c]["out"] for c in range(NCORES)], axis=0)
    return out.astype(np.float32)


# revision 9
# speedup vs baseline: 1.1193x; 1.0121x over previous
"""Trainium2 Bass kernel for the correlation-map embedding module.

Math (per (b, nf) pair):
  f1d = bilinear_down28(feature_i[b, nf])                  # [C, 28, 28]
  f2sel[c, k] = bilinear sample of feature_j[b, nf] at the K knn grid points
  corr[k, :, :] = relu(sum_c f2sel[c, k] * f1d[c, :, :])   # [K, 28, 28]
  out[k] = corr[k] / sum_hw(exp(corr[k])) * 10

Key restructurings vs the reference:
  - only the K=128 selected query positions of f2 are ever computed (4-tap
    gather on GPSIMD, tap weighting + tap reduction on DVE), so every matmul
    shares one stationary f2sel operand;
  - the f1 bilinear downsample never materializes: each input element of the
    56x56 plane contributes to exactly one 28x28 output cell with one product
    weight, so a single contiguous full-plane multiply (f1 * W4full, one DVE
    op per half) replaces the strided 4-tap mul/add tree, and the 2x2-block
    reduction folds into 4 accumulating matmuls whose rhs are the even/odd
    strided views of the weighted plane;
  - the channel contraction runs on the tensor engine in float32r;
  - epilogue on ScalarE: relu(corr)*10 via activation scale, exp via Exp with
    scale=0.1 + accum_out, normalize via DVE tensor_scalar;
  - per nf, f2 loads before f1 so the gather -> weight -> reduce chain runs
    while f1 is still streaming (the gather chain is the longest dependency).

Sharding: pure data parallel - batch dim (16) split across 8 cores, 2 each.
"""

import numpy as np

# hardcoded problem shapes (grading calls kernel(**inputs) standalone)
B, NF, C, H, W = 16, 3, 128, 56, 56
G = 28
K = 128
NCORES = 8
BPC = B // NCORES  # 2
P = 128
HH = H // 2  # 28 input rows per half
GH = G // 2  # 14 output rows per half

_CACHE = {}


def _axis_coords(n_in):
    # float32 arithmetic to match the jax reference bit-for-bit
    src = np.arange(G, dtype=np.float32) * np.float32((n_in - 1) / (G - 1))
    i0 = np.clip(np.floor(src).astype(np.int32), 0, n_in - 2)
    w = (src - i0.astype(np.float32)).astype(np.float32)
    return i0, w


def _host_consts(knn_inds):
    i0h, wh = _axis_coords(H)
    i0w, ww = _axis_coords(W)
    # the even/odd strided-AP downsample assumes taps are (2k, 2k+1)
    assert np.array_equal(i0h, 2 * np.arange(G)) and np.array_equal(i0w, 2 * np.arange(G))

    ah, bh = (1.0 - wh), wh
    aw, bw = (1.0 - ww), ww
    # full-plane product weights: input element (2h'+u, 2w'+t) belongs to
    # output cell (h', w') with weight wh_tap[u][h'] * ww_tap[t][w']
    whfull = np.empty(H, dtype=np.float32)
    whfull[0::2] = ah
    whfull[1::2] = bh
    wwfull = np.empty(W, dtype=np.float32)
    wwfull[0::2] = aw
    wwfull[1::2] = bw
    w4full = np.outer(whfull, wwfull).astype(np.float32).reshape(-1)  # [3136]

    # gather indices/weights for the 4 bilinear taps of each knn point
    knn = np.asarray(knn_inds).astype(np.int64)  # [NF, K, 2]
    gidx2 = np.zeros((NF, P, 32), dtype=np.int16)
    gwts = np.zeros((NF, 4 * K), dtype=np.float32)
    for nf in range(NF):
        h2 = knn[nf, :, 1]
        w2 = knn[nf, :, 0]
        r0 = i0h[h2]
        c0 = i0w[w2]
        # d=2 gather: each index fetches the horizontally-contiguous tap pair
        # (r_u*W + c0, +1); index is in units of 2 elements (c0 even).
        # j = k*2 + u ordering: gathered tile is [P, K, 2, 2] = [P, K, 4]
        pos = np.stack(
            [(r0 * W + c0) // 2, ((r0 + 1) * W + c0) // 2], axis=1
        ).reshape(-1)  # [256]
        wt = np.stack(
            [ah[h2] * aw[w2], ah[h2] * bw[w2], bh[h2] * aw[w2], bh[h2] * bw[w2]],
            axis=1,
        ).reshape(-1)
        gwts[nf] = wt.astype(np.float32)
        # ap_gather index layout: gathered index j comes from partition j%16,
        # slot j//16 of its 16-partition group; replicate across the 8 groups.
        # merged variant: one gather per nf covering both batches stacked in
        # one [P, 2*H*W] tile; j = b*256 + k*2 + u, b offset in d=2 units
        pos2 = np.concatenate([pos, pos + H * W // 2])  # [512]
        wrapped2 = pos2.reshape(32, 16).T.astype(np.int16)  # [16, 32]
        gidx2[nf] = np.tile(wrapped2, (8, 1))
    return w4full, gidx2, gwts


def _build_bass():
    import concourse.bacc as bacc
    import concourse.tile as tile
    from concourse import mybir

    f32 = mybir.dt.float32
    f32r = mybir.dt.float32r
    i16 = mybir.dt.int16
    AF = mybir.ActivationFunctionType
    ALU = mybir.AluOpType

    nc = bacc.Bacc()
    # fi declared f32r so the in-place weighted plane feeds the PE at full rate
    fi = nc.dram_tensor("fi", [BPC, NF, C, H, W], f32r, kind="ExternalInput")
    fj = nc.dram_tensor("fj", [BPC, NF, C, H, W], f32, kind="ExternalInput")
    w4_d = nc.dram_tensor("w4", [1, H * W + P], f32r, kind="ExternalInput")
    gidx_d = nc.dram_tensor("gidx", [NF, P, 32], i16, kind="ExternalInput")
    gw_d = nc.dram_tensor("gw", [1, NF * 4 * K], f32r, kind="ExternalInput")
    out_d = nc.dram_tensor("out", [BPC, NF, K, G, G], f32, kind="ExternalOutput")

    with tile.TileContext(nc) as tc:
        with (
            tc.tile_pool(name="consts", bufs=1) as consts,
            tc.tile_pool(name="feat2", bufs=3) as feat2,
            tc.tile_pool(name="feat1", bufs=2) as feat1,
            tc.tile_pool(name="work", bufs=2) as work,
            tc.tile_pool(name="gpool", bufs=3) as gpool,
            tc.tile_pool(name="psum", bufs=3, space="PSUM") as pspool,
            tc.tile_pool(name="bcpsum", bufs=2, space="PSUM") as bcpool,
            tc.tile_pool(name="outp", bufs=2) as outp,
            tc.tile_pool(name="opool", bufs=3) as opool,
        ):
            # dummy warmup gather on memset inputs (no DMA deps): forces the
            # GPSIMD ext-isa library load (MODIFY_POOL_CONFIG + ~6us IRAM
            # fetch) to happen during the initial DMA fill instead of
            # serializing the first real gather
            zsrc = consts.tile([P, 32], f32, tag="zsrc")
            nc.vector.memset(zsrc, 0.0)
            zidx = consts.tile([P, 1], i16, tag="zidx")
            nc.vector.memset(zidx, 0)
            gdummy = consts.tile([P, 32], f32, tag="gdummy")
            nc.gpsimd.ap_gather(
                gdummy, zsrc, zidx, channels=P, num_elems=16, d=2, num_idxs=16
            )

            # constants: tiny single-partition rows, loaded on the sync ring
            # BEFORE the feature loads so they complete in the ramp-up window
            # instead of queueing behind MBs of feature traffic. Then
            # replicate across partitions with ones-vector matmuls on the
            # idle PE + copies on the idle DVE. float32r rounding of the
            # weights (~1e-3) is in the same class as the matmul's own.
            w4row = consts.tile([1, H * W + P], f32r, tag="w4row")
            nc.sync.dma_start(out=w4row, in_=w4_d[:, :])
            gwrow = consts.tile([1, NF * 4 * K], f32r, tag="gwrow")
            nc.sync.dma_start(out=gwrow, in_=gw_d[:, :])
            gidx_t = []
            for nf in range(NF):
                it = consts.tile([P, 32], i16, tag=f"gidx{nf}")
                nc.sync.dma_start(out=it, in_=gidx_d[nf])
                gidx_t.append(it)
            # trailing P entries of the w4 input are 1.0: the ones row for
            # the PE partition-broadcast matmuls
            ones = w4row[:, H * W : H * W + P]

            bc_count = [0]

            def pe_broadcast(row_ap, n):
                """[1, n] -> [P, n] via PE: out = ones.T @ row."""
                dst = consts.tile([P, n], f32, tag=f"bc{bc_count[0]}")
                bc_count[0] += 1
                done = 0
                while done < n:
                    chunk = min(512, n - done)
                    bps = bcpool.tile([P, 512], f32, tag="bps")
                    nc.tensor.matmul(
                        bps[:, :chunk],
                        lhsT=ones,
                        rhs=row_ap[:, done : done + chunk],
                        start=True,
                        stop=True,
                    )
                    nc.vector.tensor_copy(dst[:, done : done + chunk], bps[:, :chunk])
                    done += chunk
                return dst

            w4bc = pe_broadcast(w4row[:, : H * W], H * W)  # [P, 3136]
            gw_t = [
                pe_broadcast(gwrow[:, nf * 4 * K : (nf + 1) * 4 * K], 4 * K)
                for nf in range(NF)
            ]

            for nf in range(NF):
                # f2 first: the gather -> weight -> reduce chain is the
                # longest dependency; it runs while f1 still streams
                f2x = feat2.tile([P, BPC, H * W], f32, tag="f2x")
                for b in range(BPC):
                    nc.sync.dma_start(
                        out=f2x[:, b, :],
                        in_=fj[b, nf].rearrange("p h w -> p (h w)"),
                    )
                f1h = {}
                for b in range(BPC):
                    f1h[b] = []
                    for h in range(2):
                        t = feat1.tile([P, HH, W], f32r, tag=f"f1_{b}_{h}")
                        nc.sync.dma_start(
                            out=t, in_=fi[b, nf, :, h * HH : (h + 1) * HH, :]
                        )
                        f1h[b].append(t)

                # f2 at the K selected grid points: one merged gather per nf
                # (GPSIMD), then per-b tap weights + tap reduction (DVE) so
                # all matmuls share one stationary f2sel operand
                g2 = gpool.tile([P, BPC, K, 4], f32, tag="g2")
                nc.gpsimd.ap_gather(
                    g2.rearrange("p b k t -> p (b k t)"),
                    f2x.rearrange("p b q -> p (b q)"),
                    gidx_t[nf],
                    channels=P,
                    num_elems=BPC * H * W // 2,
                    d=2,
                    num_idxs=BPC * 2 * K,
                )

                # fused tap weighting first in DVE order: these only need f1
                # and run while the gather chain waits on f2
                for b in range(BPC):
                    for h in range(2):
                        fh = f1h[b][h].rearrange("p h w -> p (h w)")
                        nc.vector.tensor_mul(
                            fh, fh, w4bc[:, h * HH * W : (h + 1) * HH * W]
                        )

                f2sel = {}
                for b in range(BPC):
                    gg = work.tile([P, K, 4], f32r, tag=f"gg{b}")
                    nc.vector.tensor_mul(
                        gg.rearrange("p k t -> p (k t)"),
                        g2[:, b].rearrange("p k t -> p (k t)"),
                        gw_t[nf],
                    )
                    fs = work.tile([P, K], f32r, tag=f"fs{b}")
                    with nc.allow_low_precision(reason="f32r is fp32-width"):
                        nc.vector.tensor_reduce(
                            fs, gg, axis=mybir.AxisListType.X, op=ALU.add
                        )
                    f2sel[b] = fs

                for b in range(BPC):
                    ps = pspool.tile([P, 2, 512], f32, tag="ps")
                    for h in range(2):
                        # 2x2-block reduction on the PE: 4 accumulating
                        # matmuls over the even/odd strided views of the
                        # weighted plane
                        f1v = f1h[b][h].rearrange(
                            "p (h uu) (w tt) -> p h uu w tt", uu=2, tt=2
                        )
                        i = 0
                        for u in range(2):
                            for t in range(2):
                                nc.tensor.matmul(
                                    ps[:, h, : GH * G],
                                    lhsT=f2sel[b],
                                    rhs=f1v[:, :, u, :, t],
                                    start=(i == 0),
                                    stop=(i == 3),
                                )
                                i += 1

                    # epilogue on ScalarE: r = 10*relu(corr);
                    # s = sum(exp(r/10)); out = r * (1/s)
                    r = outp.tile([P, 2, GH * G], f32, tag="r")
                    nc.scalar.activation(r, ps[:, :, : GH * G], AF.Relu, scale=10.0)
                    rf = r.rearrange("p h q -> p (h q)")  # [P, 784] contiguous
                    e = outp.tile([P, G * G], f32, tag="e")
                    s = work.tile([P, 1], f32, tag="s")
                    nc.scalar.activation(e, rf, AF.Exp, scale=0.1, accum_out=s)
                    rec = work.tile([P, 1], f32, tag="rec")
                    nc.vector.reciprocal(rec, s)
                    o = opool.tile([P, G * G], f32, tag="o")
                    # final normalize on DVE (tensor_scalar runs in 2x mode)
                    nc.vector.tensor_scalar(o, rf, rec, None, op0=ALU.mult)
                    # issue the store from ScalarE (mostly idle): keeps the
                    # SP/sync stream free to prefetch later pairs
                    nc.scalar.dma_start(
                        out=out_d[b, nf].rearrange("k g1 g2 -> k (g1 g2)"), in_=o
                    )
    return nc


def _get_bass():
    if "nc" not in _CACHE:
        nc = _build_bass()
        # run the Bacc passes (reg alloc, library-load insertion) before the
        # PJRT path serializes the module
        if not nc.is_finalized():
            nc.finalize()
        _CACHE["nc"] = nc
    return _CACHE["nc"]


def kernel(feature_i, feature_j, mask, optical_flow, knn_inds):
    from concourse import bass_utils

    nc = _get_bass()
    w4full, gidx, gwts = _host_consts(knn_inds)

    fi = np.ascontiguousarray(np.asarray(feature_i, dtype=np.float32))
    fj = np.ascontiguousarray(np.asarray(feature_j, dtype=np.float32))
    w4in = np.concatenate([w4full, np.ones(P, np.float32)])[None, :]

    in_maps = []
    for core in range(NCORES):
        lo = core * BPC
        in_maps.append(
            {
                "fi": fi[lo : lo + BPC],
                "fj": fj[lo : lo + BPC],
                "w4": w4in,
                "gidx": gidx,
                "gw": gwts.reshape(1, -1),
            }
        )

    res = bass_utils.run_bass_kernel_spmd(nc, in_maps, core_ids=list(range(NCORES)))
    out = np.concatenate([res.results[c]["out"] for c in range(NCORES)], axis=0)
    return out.astype(np.float32)


# revision 11
# speedup vs baseline: 1.1206x; 1.0012x over previous
"""Trainium2 Bass kernel for the correlation-map embedding module.

Math (per (b, nf) pair):
  f1d = bilinear_down28(feature_i[b, nf])                  # [C, 28, 28]
  f2sel[c, k] = bilinear sample of feature_j[b, nf] at the K knn grid points
  corr[k, :, :] = relu(sum_c f2sel[c, k] * f1d[c, :, :])   # [K, 28, 28]
  out[k] = corr[k] / sum_hw(exp(corr[k])) * 10

Key restructurings vs the reference:
  - only the K=128 selected query positions of f2 are ever computed (4-tap
    gather on GPSIMD, tap weighting + tap reduction on DVE), so every matmul
    shares one stationary f2sel operand;
  - the f1 bilinear downsample never materializes: each input element of the
    56x56 plane contributes to exactly one 28x28 output cell with one product
    weight, so a single contiguous full-plane multiply (f1 * W4full, one DVE
    op per half) replaces the strided 4-tap mul/add tree, and the 2x2-block
    reduction folds into 4 accumulating matmuls whose rhs are the even/odd
    strided views of the weighted plane;
  - the channel contraction runs on the tensor engine in float32r;
  - epilogue on ScalarE: relu(corr)*10 via activation scale, exp via Exp with
    scale=0.1 + accum_out, normalize via DVE tensor_scalar;
  - per nf, f2 loads before f1 so the gather -> weight -> reduce chain runs
    while f1 is still streaming (the gather chain is the longest dependency).

Sharding: pure data parallel - batch dim (16) split across 8 cores, 2 each.
"""

import numpy as np

# hardcoded problem shapes (grading calls kernel(**inputs) standalone)
B, NF, C, H, W = 16, 3, 128, 56, 56
G = 28
K = 128
NCORES = 8
BPC = B // NCORES  # 2
P = 128
HH = H // 2  # 28 input rows per half
GH = G // 2  # 14 output rows per half

_CACHE = {}


def _axis_coords(n_in):
    # float32 arithmetic to match the jax reference bit-for-bit
    src = np.arange(G, dtype=np.float32) * np.float32((n_in - 1) / (G - 1))
    i0 = np.clip(np.floor(src).astype(np.int32), 0, n_in - 2)
    w = (src - i0.astype(np.float32)).astype(np.float32)
    return i0, w


def _host_consts(knn_inds):
    i0h, wh = _axis_coords(H)
    i0w, ww = _axis_coords(W)
    # the even/odd strided-AP downsample assumes taps are (2k, 2k+1)
    assert np.array_equal(i0h, 2 * np.arange(G)) and np.array_equal(i0w, 2 * np.arange(G))

    ah, bh = (1.0 - wh), wh
    aw, bw = (1.0 - ww), ww
    # full-plane product weights: input element (2h'+u, 2w'+t) belongs to
    # output cell (h', w') with weight wh_tap[u][h'] * ww_tap[t][w']
    whfull = np.empty(H, dtype=np.float32)
    whfull[0::2] = ah
    whfull[1::2] = bh
    wwfull = np.empty(W, dtype=np.float32)
    wwfull[0::2] = aw
    wwfull[1::2] = bw
    w4full = np.outer(whfull, wwfull).astype(np.float32).reshape(-1)  # [3136]

    # gather indices/weights for the 4 bilinear taps of each knn point
    knn = np.asarray(knn_inds).astype(np.int64)  # [NF, K, 2]
    gidx2 = np.zeros((NF, P, 32), dtype=np.int16)
    gwts = np.zeros((NF, 4 * K), dtype=np.float32)
    for nf in range(NF):
        h2 = knn[nf, :, 1]
        w2 = knn[nf, :, 0]
        r0 = i0h[h2]
        c0 = i0w[w2]
        # d=2 gather: each index fetches the horizontally-contiguous tap pair
        # (r_u*W + c0, +1); index is in units of 2 elements (c0 even).
        # j = k*2 + u ordering: gathered tile is [P, K, 2, 2] = [P, K, 4]
        pos = np.stack(
            [(r0 * W + c0) // 2, ((r0 + 1) * W + c0) // 2], axis=1
        ).reshape(-1)  # [256]
        wt = np.stack(
            [ah[h2] * aw[w2], ah[h2] * bw[w2], bh[h2] * aw[w2], bh[h2] * bw[w2]],
            axis=1,
        ).reshape(-1)
        gwts[nf] = wt.astype(np.float32)
        # ap_gather index layout: gathered index j comes from partition j%16,
        # slot j//16 of its 16-partition group; replicate across the 8 groups.
        # merged variant: one gather per nf covering both batches stacked in
        # one [P, 2*H*W] tile; j = b*256 + k*2 + u, b offset in d=2 units
        pos2 = np.concatenate([pos, pos + H * W // 2])  # [512]
        wrapped2 = pos2.reshape(32, 16).T.astype(np.int16)  # [16, 32]
        gidx2[nf] = np.tile(wrapped2, (8, 1))
    return w4full, gidx2, gwts


def _build_bass():
    import concourse.bacc as bacc
    import concourse.tile as tile
    from concourse import mybir

    f32 = mybir.dt.float32
    f32r = mybir.dt.float32r
    bf16 = mybir.dt.bfloat16
    i16 = mybir.dt.int16
    AF = mybir.ActivationFunctionType
    ALU = mybir.AluOpType

    nc = bacc.Bacc()
    # fi declared f32r so the in-place weighted plane feeds the PE at full rate
    fi = nc.dram_tensor("fi", [BPC, NF, C, H, W], f32, kind="ExternalInput")
    fj = nc.dram_tensor("fj", [BPC, NF, C, H, W], f32, kind="ExternalInput")
    w4_d = nc.dram_tensor("w4", [1, H * W + P], f32r, kind="ExternalInput")
    gidx_d = nc.dram_tensor("gidx", [NF, P, 32], i16, kind="ExternalInput")
    gw_d = nc.dram_tensor("gw", [1, NF * 4 * K], f32r, kind="ExternalInput")
    out_d = nc.dram_tensor("out", [BPC, NF, K, G, G], f32, kind="ExternalOutput")

    with tile.TileContext(nc) as tc:
        with (
            tc.tile_pool(name="consts", bufs=1) as consts,
            tc.tile_pool(name="feat2", bufs=2) as feat2,
            tc.tile_pool(name="feat1", bufs=2) as feat1,
            tc.tile_pool(name="work", bufs=2) as work,
            tc.tile_pool(name="gpool", bufs=3) as gpool,
            tc.tile_pool(name="psum", bufs=3, space="PSUM") as pspool,
            tc.tile_pool(name="bcpsum", bufs=2, space="PSUM") as bcpool,
            tc.tile_pool(name="outp", bufs=2) as outp,
            tc.tile_pool(name="opool", bufs=3) as opool,
        ):
            # dummy warmup gather on memset inputs (no DMA deps): forces the
            # GPSIMD ext-isa library load (MODIFY_POOL_CONFIG + ~6us IRAM
            # fetch) to happen during the initial DMA fill instead of
            # serializing the first real gather
            zsrc = consts.tile([P, 32], f32, tag="zsrc")
            nc.vector.memset(zsrc, 0.0)
            zidx = consts.tile([P, 1], i16, tag="zidx")
            nc.vector.memset(zidx, 0)
            gdummy = consts.tile([P, 32], f32, tag="gdummy")
            nc.gpsimd.ap_gather(
                gdummy, zsrc, zidx, channels=P, num_elems=16, d=2, num_idxs=16
            )

            # constants: tiny single-partition rows, loaded on the sync ring
            # BEFORE the feature loads so they complete in the ramp-up window
            # instead of queueing behind MBs of feature traffic. Then
            # replicate across partitions with ones-vector matmuls on the
            # idle PE + copies on the idle DVE. float32r rounding of the
            # weights (~1e-3) is in the same class as the matmul's own.
            w4row = consts.tile([1, H * W + P], f32r, tag="w4row")
            nc.sync.dma_start(out=w4row, in_=w4_d[:, :])
            gwrow = consts.tile([1, NF * 4 * K], f32r, tag="gwrow")
            nc.sync.dma_start(out=gwrow, in_=gw_d[:, :])
            gidx_t = []
            for nf in range(NF):
                it = consts.tile([P, 32], i16, tag=f"gidx{nf}")
                nc.sync.dma_start(out=it, in_=gidx_d[nf])
                gidx_t.append(it)
            # trailing P entries of the w4 input are 1.0: the ones row for
            # the PE partition-broadcast matmuls
            ones = w4row[:, H * W : H * W + P]

            bc_count = [0]

            def pe_broadcast(row_ap, n):
                """[1, n] -> [P, n] via PE: out = ones.T @ row."""
                dst = consts.tile([P, n], f32, tag=f"bc{bc_count[0]}")
                bc_count[0] += 1
                done = 0
                while done < n:
                    chunk = min(512, n - done)
                    bps = bcpool.tile([P, 512], f32, tag="bps")
                    nc.tensor.matmul(
                        bps[:, :chunk],
                        lhsT=ones,
                        rhs=row_ap[:, done : done + chunk],
                        start=True,
                        stop=True,
                    )
                    nc.vector.tensor_copy(dst[:, done : done + chunk], bps[:, :chunk])
                    done += chunk
                return dst

            w4bc = pe_broadcast(w4row[:, : H * W], H * W)  # [P, 3136]
            gw_t = [
                pe_broadcast(gwrow[:, nf * 4 * K : (nf + 1) * 4 * K], 4 * K)
                for nf in range(NF)
            ]

            # per-nf input loads + eager gathers: the Pool queue stays
            # [dummy, g0, g1, g2] with only f2x/gidx waits, so Tile's
            # wait-hoisting cannot serialize a gather behind unrelated
            # later traffic
            f2xs, f1hs, g2s = [], {}, []
            for nf in range(NF):
                f2x = feat2.tile([P, BPC, H * W], f32, tag="f2x")
                for b in range(BPC):
                    nc.sync.dma_start(
                        out=f2x[:, b, :],
                        in_=fj[b, nf].rearrange("p h w -> p (h w)"),
                    )
                f2xs.append(f2x)
                g2 = gpool.tile([P, BPC, K, 4], f32, tag="g2")
                nc.gpsimd.ap_gather(
                    g2.rearrange("p b k t -> p (b k t)"),
                    f2x.rearrange("p b q -> p (b q)"),
                    gidx_t[nf],
                    channels=P,
                    num_elems=BPC * H * W // 2,
                    d=2,
                    num_idxs=BPC * 2 * K,
                )
                g2s.append(g2)
                for b in range(BPC):
                    f1hs[nf, b] = []
                    for h in range(2):
                        t = feat1.tile([P, HH, W], f32, tag=f"f1_{b}_{h}")
                        nc.sync.dma_start(
                            out=t, in_=fi[b, nf, :, h * HH : (h + 1) * HH, :]
                        )
                        f1hs[nf, b].append(t)

            for nf in range(NF):
                g2 = g2s[nf]
                f1h = {b: f1hs[nf, b] for b in range(BPC)}

                # fused tap weighting: writes a separate bf16 plane so the
                # f32 f1 tile recycles DVE-paced (decoupled from the PE),
                # and the matmuls run bf16 with fast weight loads
                wt = {}
                for b in range(BPC):
                    wt[b] = []
                    for h in range(2):
                        w = work.tile([P, HH, W], bf16, tag=f"wt_{b}_{h}")
                        nc.vector.tensor_mul(
                            w.rearrange("p h w -> p (h w)"),
                            f1h[b][h].rearrange("p h w -> p (h w)"),
                            w4bc[:, h * HH * W : (h + 1) * HH * W],
                        )
                        wt[b].append(w)

                f2sel = {}
                for b in range(BPC):
                    gg = work.tile([P, K, 4], f32r, tag=f"gg{b}")
                    nc.vector.tensor_mul(
                        gg.rearrange("p k t -> p (k t)"),
                        g2[:, b].rearrange("p k t -> p (k t)"),
                        gw_t[nf],
                    )
                    fs = work.tile([P, K], bf16, tag=f"fs{b}")
                    with nc.allow_low_precision(reason="bf16 products, fp32 psum"):
                        nc.vector.tensor_reduce(
                            fs, gg, axis=mybir.AxisListType.X, op=ALU.add
                        )
                    f2sel[b] = fs

                for b in range(BPC):
                    ps = pspool.tile([P, 2, 512], f32, tag="ps")
                    for h in range(2):
                        # 2x2-block reduction on the PE: 4 accumulating
                        # matmuls over the even/odd strided views of the
                        # weighted plane
                        f1v = wt[b][h].rearrange(
                            "p (h uu) (w tt) -> p h uu w tt", uu=2, tt=2
                        )
                        i = 0
                        for u in range(2):
                            for t in range(2):
                                nc.tensor.matmul(
                                    ps[:, h, : GH * G],
                                    lhsT=f2sel[b],
                                    rhs=f1v[:, :, u, :, t],
                                    start=(i == 0),
                                    stop=(i == 3),
                                )
                                i += 1

                    # epilogue on ScalarE: r = 10*relu(corr);
                    # s = sum(exp(r/10)); out = r * (1/s)
                    r = outp.tile([P, 2, GH * G], f32, tag="r")
                    nc.scalar.activation(r, ps[:, :, : GH * G], AF.Relu, scale=10.0)
                    rf = r.rearrange("p h q -> p (h q)")  # [P, 784] contiguous
                    e = outp.tile([P, G * G], f32, tag="e")
                    s = work.tile([P, 1], f32, tag="s")
                    nc.scalar.activation(e, rf, AF.Exp, scale=0.1, accum_out=s)
                    rec = work.tile([P, 1], f32, tag="rec")
                    nc.vector.reciprocal(rec, s)
                    o = opool.tile([P, G * G], f32, tag="o")
                    # final normalize on DVE (tensor_scalar runs in 2x mode)
                    nc.vector.tensor_scalar(o, rf, rec, None, op0=ALU.mult)
                    # issue the store from ScalarE (mostly idle): keeps the
                    # SP/sync stream free to prefetch later pairs
                    nc.scalar.dma_start(
                        out=out_d[b, nf].rearrange("k g1 g2 -> k (g1 g2)"), in_=o
                    )
    return nc


def _get_bass():
    if "nc" not in _CACHE:
        nc = _build_bass()
        # run the Bacc passes (reg alloc, library-load insertion) before the
        # PJRT path serializes the module
        if not nc.is_finalized():
            nc.finalize()
        _CACHE["nc"] = nc
    return _CACHE["nc"]


def kernel(feature_i, feature_j, mask, optical_flow, knn_inds):
    from concourse import bass_utils

    nc = _get_bass()
    w4full, gidx, gwts = _host_consts(knn_inds)

    fi = np.ascontiguousarray(np.asarray(feature_i, dtype=np.float32))
    fj = np.ascontiguousarray(np.asarray(feature_j, dtype=np.float32))
    w4in = np.concatenate([w4full, np.ones(P, np.float32)])[None, :]

    in_maps = []
    for core in range(NCORES):
        lo = core * BPC
        in_maps.append(
            {
                "fi": fi[lo : lo + BPC],
                "fj": fj[lo : lo + BPC],
                "w4": w4in,
                "gidx": gidx,
                "gw": gwts.reshape(1, -1),
            }
        )

    res = bass_utils.run_bass_kernel_spmd(nc, in_maps, core_ids=list(range(NCORES)))
    out = np.concatenate([res.results[c]["out"] for c in range(NCORES)], axis=0)
    return out.astype(np.float32)


# revision 14
# speedup vs baseline: 1.2015x; 1.0722x over previous
"""Trainium2 Bass kernel for the correlation-map embedding module.

Math (per (b, nf) pair):
  f1d = bilinear_down28(feature_i[b, nf])                  # [C, 28, 28]
  f2sel[c, k] = bilinear sample of feature_j[b, nf] at the K knn grid points
  corr[k, :, :] = relu(sum_c f2sel[c, k] * f1d[c, :, :])   # [K, 28, 28]
  out[k] = corr[k] / sum_hw(exp(corr[k])) * 10

Key restructurings vs the reference:
  - only the K=128 selected query positions of f2 are ever computed (4-tap
    gather on GPSIMD, tap weighting + tap reduction on DVE), so every matmul
    shares one stationary f2sel operand;
  - the f1 bilinear downsample never materializes: each input element of the
    56x56 plane contributes to exactly one 28x28 output cell with one product
    weight, so a single contiguous full-plane multiply (f1 * W4full, one DVE
    op per half) replaces the strided 4-tap mul/add tree, and the 2x2-block
    reduction folds into 4 accumulating matmuls whose rhs are the even/odd
    strided views of the weighted plane;
  - the channel contraction runs on the tensor engine in float32r;
  - epilogue on ScalarE: relu(corr)*10 via activation scale, exp via Exp with
    scale=0.1 + accum_out, normalize via DVE tensor_scalar;
  - per nf, f2 loads before f1 so the gather -> weight -> reduce chain runs
    while f1 is still streaming (the gather chain is the longest dependency).

Sharding: pure data parallel - batch dim (16) split across 8 cores, 2 each.
"""

import numpy as np

# hardcoded problem shapes (grading calls kernel(**inputs) standalone)
B, NF, C, H, W = 16, 3, 128, 56, 56
G = 28
K = 128
NCORES = 8
BPC = B // NCORES  # 2
P = 128
HH = H // 2  # 28 input rows per half
GH = G // 2  # 14 output rows per half

_CACHE = {}


def _axis_coords(n_in):
    # float32 arithmetic to match the jax reference bit-for-bit
    src = np.arange(G, dtype=np.float32) * np.float32((n_in - 1) / (G - 1))
    i0 = np.clip(np.floor(src).astype(np.int32), 0, n_in - 2)
    w = (src - i0.astype(np.float32)).astype(np.float32)
    return i0, w


def _host_consts(knn_inds):
    i0h, wh = _axis_coords(H)
    i0w, ww = _axis_coords(W)
    # the even/odd strided-AP downsample assumes taps are (2k, 2k+1)
    assert np.array_equal(i0h, 2 * np.arange(G)) and np.array_equal(i0w, 2 * np.arange(G))

    ah, bh = (1.0 - wh), wh
    aw, bw = (1.0 - ww), ww
    # full-plane product weights: input element (2h'+u, 2w'+t) belongs to
    # output cell (h', w') with weight wh_tap[u][h'] * ww_tap[t][w']
    whfull = np.empty(H, dtype=np.float32)
    whfull[0::2] = ah
    whfull[1::2] = bh
    wwfull = np.empty(W, dtype=np.float32)
    wwfull[0::2] = aw
    wwfull[1::2] = bw
    w4full = np.outer(whfull, wwfull).astype(np.float32).reshape(-1)  # [3136]

    # gather indices/weights for the 4 bilinear taps of each knn point.
    # the f2 plane is pre-interleaved on-chip into rpp[c, p2, 4] (the 2x2
    # patch of raw-f2 taps for downsampled cell p2 stored contiguously), so
    # one d=4 gather index per knn point fetches its whole patch - the
    # GPSIMD ap_gather cost is per-index, so this halves it
    knn = np.asarray(knn_inds).astype(np.int64)  # [NF, K, 2]
    gidx = np.zeros((NF, P, K // 16), dtype=np.int16)
    gwts = np.zeros((NF, 4 * K), dtype=np.float32)
    for nf in range(NF):
        h2 = knn[nf, :, 1]
        w2 = knn[nf, :, 0]
        pos = (h2 * G + w2).astype(np.int64)  # [K] patch index, units of d=4
        wt = np.stack(
            [ah[h2] * aw[w2], ah[h2] * bw[w2], bh[h2] * aw[w2], bh[h2] * bw[w2]],
            axis=1,
        ).reshape(-1)
        gwts[nf] = wt.astype(np.float32)
        # ap_gather index layout: gathered index j comes from partition j%16,
        # slot j//16 of its 16-partition group; replicate across the 8 groups
        wrapped = pos.reshape(K // 16, 16).T.astype(np.int16)  # [16, 8]
        gidx[nf] = np.tile(wrapped, (8, 1))
    return w4full, gidx, gwts


def _build_bass():
    import concourse.bacc as bacc
    import concourse.tile as tile
    from concourse import mybir

    f32 = mybir.dt.float32
    f32r = mybir.dt.float32r
    bf16 = mybir.dt.bfloat16
    i16 = mybir.dt.int16
    AF = mybir.ActivationFunctionType
    ALU = mybir.AluOpType

    nc = bacc.Bacc()
    # fi declared f32r so the in-place weighted plane feeds the PE at full rate
    fi = nc.dram_tensor("fi", [BPC, NF, C, H, W], f32r, kind="ExternalInput")
    fj = nc.dram_tensor("fj", [BPC, NF, C, H, W], f32, kind="ExternalInput")
    w4_d = nc.dram_tensor("w4", [1, H * W + P], f32r, kind="ExternalInput")
    gidx_d = nc.dram_tensor("gidx", [NF, P, K // 16], i16, kind="ExternalInput")
    gw_d = nc.dram_tensor("gw", [1, NF * 4 * K], f32r, kind="ExternalInput")
    out_d = nc.dram_tensor("out", [BPC, NF, K, G, G], f32, kind="ExternalOutput")

    with tile.TileContext(nc) as tc:
        with (
            tc.tile_pool(name="consts", bufs=1) as consts,
            tc.tile_pool(name="feat2", bufs=2) as feat2,
            tc.tile_pool(name="feat1", bufs=2) as feat1,
            tc.tile_pool(name="work", bufs=2) as work,
            tc.tile_pool(name="gpool", bufs=3) as gpool,
            tc.tile_pool(name="psum", bufs=3, space="PSUM") as pspool,
            tc.tile_pool(name="bcpsum", bufs=2, space="PSUM") as bcpool,
            tc.tile_pool(name="outp", bufs=2) as outp,
            tc.tile_pool(name="opool", bufs=3) as opool,
        ):
            # dummy warmup gather on memset inputs (no DMA deps): forces the
            # GPSIMD ext-isa library load (MODIFY_POOL_CONFIG + ~6us IRAM
            # fetch) to happen during the initial DMA fill instead of
            # serializing the first real gather
            zsrc = consts.tile([P, 32], f32, tag="zsrc")
            nc.vector.memset(zsrc, 0.0)
            zidx = consts.tile([P, 1], i16, tag="zidx")
            nc.vector.memset(zidx, 0)
            gdummy = consts.tile([P, 32], f32, tag="gdummy")
            nc.gpsimd.ap_gather(
                gdummy, zsrc, zidx, channels=P, num_elems=16, d=2, num_idxs=16
            )

            # constants: tiny single-partition rows, loaded on the sync ring
            # BEFORE the feature loads so they complete in the ramp-up window
            # instead of queueing behind MBs of feature traffic. Then
            # replicate across partitions with ones-vector matmuls on the
            # idle PE + copies on the idle DVE. float32r rounding of the
            # weights (~1e-3) is in the same class as the matmul's own.
            w4row = consts.tile([1, H * W + P], f32r, tag="w4row")
            nc.sync.dma_start(out=w4row, in_=w4_d[:, :])
            gwrow = consts.tile([1, NF * 4 * K], f32r, tag="gwrow")
            nc.sync.dma_start(out=gwrow, in_=gw_d[:, :])
            gidx_t = []
            for nf in range(NF):
                it = consts.tile([P, K // 16], i16, tag=f"gidx{nf}")
                nc.sync.dma_start(out=it, in_=gidx_d[nf])
                gidx_t.append(it)
            # trailing P entries of the w4 input are 1.0: the ones row for
            # the PE partition-broadcast matmuls
            ones = w4row[:, H * W : H * W + P]

            bc_count = [0]

            def pe_broadcast(row_ap, n):
                """[1, n] -> [P, n] via PE: out = ones.T @ row."""
                dst = consts.tile([P, n], f32, tag=f"bc{bc_count[0]}")
                bc_count[0] += 1
                done = 0
                while done < n:
                    chunk = min(512, n - done)
                    bps = bcpool.tile([P, 512], f32, tag="bps")
                    nc.tensor.matmul(
                        bps[:, :chunk],
                        lhsT=ones,
                        rhs=row_ap[:, done : done + chunk],
                        start=True,
                        stop=True,
                    )
                    nc.vector.tensor_copy(dst[:, done : done + chunk], bps[:, :chunk])
                    done += chunk
                return dst

            w4bc = pe_broadcast(w4row[:, : H * W], H * W)  # [P, 3136]
            gw_t = [
                pe_broadcast(gwrow[:, nf * 4 * K : (nf + 1) * 4 * K], 4 * K)
                for nf in range(NF)
            ]

            for nf in range(NF):
                # f2 first: the interleave -> gather -> weight chain is the
                # longest dependency; it runs while f1 still streams
                f2x = feat2.tile([P, BPC, H * W], f32, tag="f2x")
                for b in range(BPC):
                    nc.sync.dma_start(
                        out=f2x[:, b, :],
                        in_=fj[b, nf].rearrange("p h w -> p (h w)"),
                    )
                f1h = {}
                for b in range(BPC):
                    f1h[b] = []
                    for h in range(2):
                        t = feat1.tile([P, HH, W], f32r, tag=f"f1_{b}_{h}")
                        nc.sync.dma_start(
                            out=t, in_=fi[b, nf, :, h * HH : (h + 1) * HH, :]
                        )
                        f1h[b].append(t)

                # pre-interleave f2 into patch-contiguous bf16 layout
                # rpp[c, b, p2, (u,t)] so one d=4 index gathers a whole 2x2
                # patch (halves the per-index-priced ap_gather); split the
                # 4 strided copies across the two otherwise-idle copy engines
                rpp = gpool.tile([P, BPC, G * G, 4], bf16, tag="rpp")
                for b in range(BPC):
                    sv = f2x[:, b].rearrange(
                        "p (h uu w tt) -> p h uu w tt", h=G, uu=2, tt=2
                    )
                    dv = rpp[:, b].rearrange("p (h w) (uu tt) -> p h w uu tt", h=G, uu=2)
                    nc.scalar.copy(dv[:, :, :, 0, :], sv[:, :, 0, :, :])
                    nc.vector.tensor_copy(dv[:, :, :, 1, :], sv[:, :, 1, :, :])

                # per-b gathers (GPSIMD), then tap weights + tap reduction
                # (DVE) so all matmuls share one stationary f2sel operand
                g2 = {}
                for b in range(BPC):
                    g = gpool.tile([P, K, 4], bf16, tag=f"g{b}")
                    nc.gpsimd.ap_gather(
                        g.rearrange("p k t -> p (k t)"),
                        rpp[:, b].rearrange("p q t -> p (q t)"),
                        gidx_t[nf],
                        channels=P,
                        num_elems=G * G,
                        d=4,
                        num_idxs=K,
                    )
                    g2[b] = g

                # fused tap weighting: one contiguous in-place mul replaces
                # the 4-tap strided mul/add tree; runs while gathers proceed
                for b in range(BPC):
                    for h in range(2):
                        fh = f1h[b][h].rearrange("p h w -> p (h w)")
                        nc.vector.tensor_mul(
                            fh, fh, w4bc[:, h * HH * W : (h + 1) * HH * W]
                        )

                f2sel = {}
                for b in range(BPC):
                    gg = work.tile([P, K, 4], f32r, tag="gg")
                    nc.vector.tensor_mul(
                        gg.rearrange("p k t -> p (k t)"),
                        g2[b].rearrange("p k t -> p (k t)"),
                        gw_t[nf],
                    )
                    fs = work.tile([P, K], f32r, tag=f"fs{b}")
                    with nc.allow_low_precision(reason="f32r is fp32-width"):
                        nc.vector.tensor_reduce(
                            fs, gg, axis=mybir.AxisListType.X, op=ALU.add
                        )
                    f2sel[b] = fs

                for b in range(BPC):
                    ps = pspool.tile([P, 2, 512], f32, tag="ps")
                    for h in range(2):
                        # 2x2-block reduction on the PE: 4 accumulating
                        # matmuls over the even/odd strided views of the
                        # weighted plane
                        f1v = f1h[b][h].rearrange(
                            "p (h uu) (w tt) -> p h uu w tt", uu=2, tt=2
                        )
                        i = 0
                        for u in range(2):
                            for t in range(2):
                                nc.tensor.matmul(
                                    ps[:, h, : GH * G],
                                    lhsT=f2sel[b],
                                    rhs=f1v[:, :, u, :, t],
                                    start=(i == 0),
                                    stop=(i == 3),
                                )
                                i += 1

                    # epilogue on ScalarE: r = 10*relu(corr);
                    # s = sum(exp(r/10)); out = r * (1/s)
                    r = outp.tile([P, 2, GH * G], f32, tag="r")
                    nc.scalar.activation(r, ps[:, :, : GH * G], AF.Relu, scale=10.0)
                    rf = r.rearrange("p h q -> p (h q)")  # [P, 784] contiguous
                    e = outp.tile([P, G * G], f32, tag="e")
                    s = work.tile([P, 1], f32, tag="s")
                    nc.scalar.activation(e, rf, AF.Exp, scale=0.1, accum_out=s)
                    rec = work.tile([P, 1], f32, tag="rec")
                    nc.vector.reciprocal(rec, s)
                    o = opool.tile([P, G * G], f32, tag="o")
                    # final normalize on DVE (tensor_scalar runs in 2x mode)
                    nc.vector.tensor_scalar(o, rf, rec, None, op0=ALU.mult)
                    # issue the store from ScalarE (mostly idle): keeps the
                    # SP/sync stream free to prefetch later pairs
                    nc.scalar.dma_start(
                        out=out_d[b, nf].rearrange("k g1 g2 -> k (g1 g2)"), in_=o
                    )
    return nc


def _get_bass():
    if "nc" not in _CACHE:
        nc = _build_bass()
        # run the Bacc passes (reg alloc, library-load insertion) before the
        # PJRT path serializes the module
        if not nc.is_finalized():
            nc.finalize()
        _CACHE["nc"] = nc
    return _CACHE["nc"]


def kernel(feature_i, feature_j, mask, optical_flow, knn_inds):
    from concourse import bass_utils

    nc = _get_bass()
    w4full, gidx, gwts = _host_consts(knn_inds)

    fi = np.ascontiguousarray(np.asarray(feature_i, dtype=np.float32))
    fj = np.ascontiguousarray(np.asarray(feature_j, dtype=np.float32))
    w4in = np.concatenate([w4full, np.ones(P, np.float32)])[None, :]

    in_maps = []
    for core in range(NCORES):
        lo = core * BPC
        in_maps.append(
            {
                "fi": fi[lo : lo + BPC],
                "fj": fj[lo : lo + BPC],
                "w4": w4in,
                "gidx": gidx,
                "gw": gwts.reshape(1, -1),
            }
        )

    res = bass_utils.run_bass_kernel_spmd(nc, in_maps, core_ids=list(range(NCORES)))
    out = np.concatenate([res.results[c]["out"] for c in range(NCORES)], axis=0)
    return out.astype(np.float32)


# revision 15
# speedup vs baseline: 1.2555x; 1.0449x over previous
"""Trainium2 Bass kernel for the correlation-map embedding module.

Math (per (b, nf) pair):
  f1d = bilinear_down28(feature_i[b, nf])                  # [C, 28, 28]
  f2sel[c, k] = bilinear sample of feature_j[b, nf] at the K knn grid points
  corr[k, :, :] = relu(sum_c f2sel[c, k] * f1d[c, :, :])   # [K, 28, 28]
  out[k] = corr[k] / sum_hw(exp(corr[k])) * 10

Key restructurings vs the reference:
  - only the K=128 selected query positions of f2 are ever computed (4-tap
    gather on GPSIMD, tap weighting + tap reduction on DVE), so every matmul
    shares one stationary f2sel operand;
  - the f1 bilinear downsample never materializes: each input element of the
    56x56 plane contributes to exactly one 28x28 output cell with one product
    weight, so a single contiguous full-plane multiply (f1 * W4full, one DVE
    op per half) replaces the strided 4-tap mul/add tree, and the 2x2-block
    reduction folds into 4 accumulating matmuls whose rhs are the even/odd
    strided views of the weighted plane;
  - the channel contraction runs on the tensor engine in float32r;
  - epilogue on ScalarE: relu(corr)*10 via activation scale, exp via Exp with
    scale=0.1 + accum_out, normalize via DVE tensor_scalar;
  - per nf, f2 loads before f1 so the gather -> weight -> reduce chain runs
    while f1 is still streaming (the gather chain is the longest dependency).

Sharding: pure data parallel - batch dim (16) split across 8 cores, 2 each.
"""

import numpy as np

# hardcoded problem shapes (grading calls kernel(**inputs) standalone)
B, NF, C, H, W = 16, 3, 128, 56, 56
G = 28
K = 128
NCORES = 8
BPC = B // NCORES  # 2
P = 128
HH = H // 2  # 28 input rows per half
GH = G // 2  # 14 output rows per half

_CACHE = {}


def _axis_coords(n_in):
    # float32 arithmetic to match the jax reference bit-for-bit
    src = np.arange(G, dtype=np.float32) * np.float32((n_in - 1) / (G - 1))
    i0 = np.clip(np.floor(src).astype(np.int32), 0, n_in - 2)
    w = (src - i0.astype(np.float32)).astype(np.float32)
    return i0, w


def _host_consts(knn_inds):
    i0h, wh = _axis_coords(H)
    i0w, ww = _axis_coords(W)
    # the even/odd strided-AP downsample assumes taps are (2k, 2k+1)
    assert np.array_equal(i0h, 2 * np.arange(G)) and np.array_equal(i0w, 2 * np.arange(G))

    ah, bh = (1.0 - wh), wh
    aw, bw = (1.0 - ww), ww
    # full-plane product weights: input element (2h'+u, 2w'+t) belongs to
    # output cell (h', w') with weight wh_tap[u][h'] * ww_tap[t][w']
    whfull = np.empty(H, dtype=np.float32)
    whfull[0::2] = ah
    whfull[1::2] = bh
    wwfull = np.empty(W, dtype=np.float32)
    wwfull[0::2] = aw
    wwfull[1::2] = bw
    w4full = np.outer(whfull, wwfull).astype(np.float32).reshape(-1)  # [3136]

    # gather indices/weights for the 4 bilinear taps of each knn point.
    # the f2 plane is pre-interleaved on-chip into rpp[c, p2, 4] (the 2x2
    # patch of raw-f2 taps for downsampled cell p2 stored contiguously), so
    # one d=4 gather index per knn point fetches its whole patch - the
    # GPSIMD ap_gather cost is per-index, so this halves it
    knn = np.asarray(knn_inds).astype(np.int64)  # [NF, K, 2]
    gidx = np.zeros((NF, P, K // 16), dtype=np.int16)
    gwts = np.zeros((NF, 4 * K), dtype=np.float32)
    for nf in range(NF):
        h2 = knn[nf, :, 1]
        w2 = knn[nf, :, 0]
        pos = (h2 * G + w2).astype(np.int64)  # [K] patch index, units of d=4
        wt = np.stack(
            [ah[h2] * aw[w2], ah[h2] * bw[w2], bh[h2] * aw[w2], bh[h2] * bw[w2]],
            axis=1,
        ).reshape(-1)
        gwts[nf] = wt.astype(np.float32)
        # ap_gather index layout: gathered index j comes from partition j%16,
        # slot j//16 of its 16-partition group; replicate across the 8 groups
        wrapped = pos.reshape(K // 16, 16).T.astype(np.int16)  # [16, 8]
        gidx[nf] = np.tile(wrapped, (8, 1))
    return w4full, gidx, gwts


def _build_bass():
    import concourse.bacc as bacc
    import concourse.tile as tile
    from concourse import mybir

    f32 = mybir.dt.float32
    f32r = mybir.dt.float32r
    bf16 = mybir.dt.bfloat16
    i16 = mybir.dt.int16
    AF = mybir.ActivationFunctionType
    ALU = mybir.AluOpType

    nc = bacc.Bacc()
    # fi declared f32r so the in-place weighted plane feeds the PE at full rate
    fi = nc.dram_tensor("fi", [BPC, NF, C, H, W], f32r, kind="ExternalInput")
    fj = nc.dram_tensor("fj", [BPC, NF, C, H, W], f32, kind="ExternalInput")
    w4_d = nc.dram_tensor("w4", [1, H * W + P], f32r, kind="ExternalInput")
    gidx_d = nc.dram_tensor("gidx", [NF, P, K // 16], i16, kind="ExternalInput")
    gw_d = nc.dram_tensor("gw", [1, NF * 4 * K], f32r, kind="ExternalInput")
    out_d = nc.dram_tensor("out", [BPC, NF, K, G, G], f32, kind="ExternalOutput")

    with tile.TileContext(nc) as tc:
        with (
            tc.tile_pool(name="consts", bufs=1) as consts,
            tc.tile_pool(name="feat2", bufs=2) as feat2,
            tc.tile_pool(name="feat1", bufs=2) as feat1,
            tc.tile_pool(name="work", bufs=2) as work,
            tc.tile_pool(name="gpool", bufs=3) as gpool,
            tc.tile_pool(name="psum", bufs=3, space="PSUM") as pspool,
            tc.tile_pool(name="bcpsum", bufs=2, space="PSUM") as bcpool,
            tc.tile_pool(name="outp", bufs=2) as outp,
            tc.tile_pool(name="opool", bufs=3) as opool,
        ):
            # dummy warmup gather on memset inputs (no DMA deps): forces the
            # GPSIMD ext-isa library load (MODIFY_POOL_CONFIG + ~6us IRAM
            # fetch) to happen during the initial DMA fill instead of
            # serializing the first real gather
            zsrc = consts.tile([P, 32], f32, tag="zsrc")
            nc.vector.memset(zsrc, 0.0)
            zidx = consts.tile([P, 1], i16, tag="zidx")
            nc.vector.memset(zidx, 0)
            gdummy = consts.tile([P, 32], f32, tag="gdummy")
            nc.gpsimd.ap_gather(
                gdummy, zsrc, zidx, channels=P, num_elems=16, d=2, num_idxs=16
            )

            # constants: tiny single-partition rows, loaded on the sync ring
            # BEFORE the feature loads so they complete in the ramp-up window
            # instead of queueing behind MBs of feature traffic. Then
            # replicate across partitions with ones-vector matmuls on the
            # idle PE + copies on the idle DVE. float32r rounding of the
            # weights (~1e-3) is in the same class as the matmul's own.
            w4row = consts.tile([1, H * W + P], f32r, tag="w4row")
            nc.sync.dma_start(out=w4row, in_=w4_d[:, :])
            gwrow = consts.tile([1, NF * 4 * K], f32r, tag="gwrow")
            nc.sync.dma_start(out=gwrow, in_=gw_d[:, :])
            gidx_t = []
            for nf in range(NF):
                it = consts.tile([P, K // 16], i16, tag=f"gidx{nf}")
                nc.sync.dma_start(out=it, in_=gidx_d[nf])
                gidx_t.append(it)
            # trailing P entries of the w4 input are 1.0: the ones row for
            # the PE partition-broadcast matmuls
            ones = w4row[:, H * W : H * W + P]

            bc_count = [0]

            def pe_broadcast(row_ap, n):
                """[1, n] -> [P, n] via PE: out = ones.T @ row."""
                dst = consts.tile([P, n], f32, tag=f"bc{bc_count[0]}")
                bc_count[0] += 1
                done = 0
                while done < n:
                    chunk = min(512, n - done)
                    bps = bcpool.tile([P, 512], f32, tag="bps")
                    nc.tensor.matmul(
                        bps[:, :chunk],
                        lhsT=ones,
                        rhs=row_ap[:, done : done + chunk],
                        start=True,
                        stop=True,
                    )
                    nc.scalar.copy(dst[:, done : done + chunk], bps[:, :chunk])
                    done += chunk
                return dst

            w4bc = pe_broadcast(w4row[:, : H * W], H * W)  # [P, 3136]
            gw_t = [
                pe_broadcast(gwrow[:, nf * 4 * K : (nf + 1) * 4 * K], 4 * K)
                for nf in range(NF)
            ]

            # prepass: per nf emit [f2x DMAs -> interleave -> gathers] with
            # no epilogue work in between, so the in-order ScalarE/Pool
            # queues never serialize an interleave or gather behind an
            # earlier nf's epilogue. DMA order front-loads f2x: the gather
            # chain for nf is ready well before its f1 arrives.
            def emit_f2_chain(nf):
                f2x = feat2.tile([P, BPC, H * W], f32, tag="f2x")
                for b in range(BPC):
                    nc.sync.dma_start(
                        out=f2x[:, b, :],
                        in_=fj[b, nf].rearrange("p h w -> p (h w)"),
                    )
                # pre-interleave f2 into patch-contiguous bf16 layout
                # rpp[c, b, p2, (u,t)] so one d=4 index gathers a whole 2x2
                # patch (halves the per-index-priced ap_gather)
                rpp = gpool.tile([P, BPC, G * G, 4], bf16, tag="rpp")
                for b in range(BPC):
                    sv = f2x[:, b].rearrange(
                        "p (h uu w tt) -> p h uu w tt", h=G, uu=2, tt=2
                    )
                    dv = rpp[:, b].rearrange("p (h w) (uu tt) -> p h w uu tt", h=G, uu=2)
                    nc.scalar.copy(dv[:, :, :, 0, :], sv[:, :, 0, :, :])
                    nc.scalar.copy(dv[:, :, :, 1, :], sv[:, :, 1, :, :])
                g2 = {}
                for b in range(BPC):
                    g = gpool.tile([P, K, 4], bf16, tag=f"g{b}")
                    nc.gpsimd.ap_gather(
                        g.rearrange("p k t -> p (k t)"),
                        rpp[:, b].rearrange("p q t -> p (q t)"),
                        gidx_t[nf],
                        channels=P,
                        num_elems=G * G,
                        d=4,
                        num_idxs=K,
                    )
                    g2[b] = g
                return g2

            def emit_f1_loads(nf):
                f1h = {}
                for b in range(BPC):
                    f1h[b] = []
                    for h in range(2):
                        t = feat1.tile([P, HH, W], f32r, tag=f"f1_{b}_{h}")
                        nc.sync.dma_start(
                            out=t, in_=fi[b, nf, :, h * HH : (h + 1) * HH, :]
                        )
                        f1h[b].append(t)
                return f1h

            g2s, f1hs = {}, {}
            g2s[0] = emit_f2_chain(0)
            g2s[1] = emit_f2_chain(1)
            f1hs[0] = emit_f1_loads(0)
            g2s[2] = emit_f2_chain(2)
            f1hs[1] = emit_f1_loads(1)
            f1hs[2] = emit_f1_loads(2)

            for nf in range(NF):
                g2 = g2s[nf]
                f1h = f1hs[nf]

                # fused tap weighting: one contiguous in-place mul replaces
                # the 4-tap strided mul/add tree; runs while gathers proceed
                for b in range(BPC):
                    for h in range(2):
                        fh = f1h[b][h].rearrange("p h w -> p (h w)")
                        nc.vector.tensor_mul(
                            fh, fh, w4bc[:, h * HH * W : (h + 1) * HH * W]
                        )

                f2sel = {}
                for b in range(BPC):
                    gg = work.tile([P, K, 4], f32r, tag="gg")
                    nc.vector.tensor_mul(
                        gg.rearrange("p k t -> p (k t)"),
                        g2[b].rearrange("p k t -> p (k t)"),
                        gw_t[nf],
                    )
                    fs = work.tile([P, K], f32r, tag=f"fs{b}")
                    with nc.allow_low_precision(reason="f32r is fp32-width"):
                        nc.vector.tensor_reduce(
                            fs, gg, axis=mybir.AxisListType.X, op=ALU.add
                        )
                    f2sel[b] = fs

                for b in range(BPC):
                    ps = pspool.tile([P, 2, 512], f32, tag="ps")
                    for h in range(2):
                        # 2x2-block reduction on the PE: 4 accumulating
                        # matmuls over the even/odd strided views of the
                        # weighted plane
                        f1v = f1h[b][h].rearrange(
                            "p (h uu) (w tt) -> p h uu w tt", uu=2, tt=2
                        )
                        i = 0
                        for u in range(2):
                            for t in range(2):
                                nc.tensor.matmul(
                                    ps[:, h, : GH * G],
                                    lhsT=f2sel[b],
                                    rhs=f1v[:, :, u, :, t],
                                    start=(i == 0),
                                    stop=(i == 3),
                                )
                                i += 1

                    # epilogue on ScalarE: r = 10*relu(corr);
                    # s = sum(exp(r/10)); out = r * (1/s)
                    r = outp.tile([P, 2, GH * G], f32, tag="r")
                    nc.scalar.activation(r, ps[:, :, : GH * G], AF.Relu, scale=10.0)
                    rf = r.rearrange("p h q -> p (h q)")  # [P, 784] contiguous
                    e = outp.tile([P, G * G], f32, tag="e")
                    s = work.tile([P, 1], f32, tag="s")
                    nc.scalar.activation(e, rf, AF.Exp, scale=0.1, accum_out=s)
                    rec = work.tile([P, 1], f32, tag="rec")
                    nc.vector.reciprocal(rec, s)
                    o = opool.tile([P, G * G], f32, tag="o")
                    # final normalize on ScalarE: Copy with per-partition scale
                    nc.scalar.activation(o, rf, AF.Copy, scale=rec)
                    # issue the store from ScalarE (mostly idle): keeps the
                    # SP/sync stream free to prefetch later pairs
                    nc.scalar.dma_start(
                        out=out_d[b, nf].rearrange("k g1 g2 -> k (g1 g2)"), in_=o
                    )
    return nc


def _get_bass():
    if "nc" not in _CACHE:
        nc = _build_bass()
        # run the Bacc passes (reg alloc, library-load insertion) before the
        # PJRT path serializes the module
        if not nc.is_finalized():
            nc.finalize()
        _CACHE["nc"] = nc
    return _CACHE["nc"]


def kernel(feature_i, feature_j, mask, optical_flow, knn_inds):
    from concourse import bass_utils

    nc = _get_bass()
    w4full, gidx, gwts = _host_consts(knn_inds)

    fi = np.ascontiguousarray(np.asarray(feature_i, dtype=np.float32))
    fj = np.ascontiguousarray(np.asarray(feature_j, dtype=np.float32))
    w4in = np.concatenate([w4full, np.ones(P, np.float32)])[None, :]

    in_maps = []
    for core in range(NCORES):
        lo = core * BPC
        in_maps.append(
            {
                "fi": fi[lo : lo + BPC],
                "fj": fj[lo : lo + BPC],
                "w4": w4in,
                "gidx": gidx,
                "gw": gwts.reshape(1, -1),
            }
        )

    res = bass_utils.run_bass_kernel_spmd(nc, in_maps, core_ids=list(range(NCORES)))
    out = np.concatenate([res.results[c]["out"] for c in range(NCORES)], axis=0)
    return out.astype(np.float32)
